# revision 19
# baseline (speedup 1.0000x reference)
"""Trainium2 Bass kernel for nn_BaselineAttnDecoder.

Data-parallel over 8 NeuronCores: each core handles 160 decode rows
(= 16 images x 10 rounds). All weights replicated.

Feature-major GRU: weights are the stationary matmul operand, the 160
batch rows stream as moving columns, so gates land directly in the
transposed layout the next step needs (no per-step h transposes).
Image attention is folded: P = (W_ih_ic @ iv_W) @ img^T and
P2 = (W_out_ic @ iv_W) @ img^T are computed once on device, so the
image context vector is never materialized; its bias contribution
(softmax weights sum to 1) is folded into gate/output biases on host.
Sigmoid is computed as 0.5 + 0.5*tanh(x/2) (with W_hh_n pre-halved) so
the whole kernel uses one activation table (exp_and_others).
The transposed embedding table for the step-19 logits is persisted in
SBUF (loaded during the encoder), with bf16 top-8 + exact f32 rescore
for the argmax re-embedding.
"""
import numpy as np
import ml_dtypes

import concourse.bass as bass
import concourse.bacc as bacc
import concourse.mybir as mybir
import concourse.tile as tile
from concourse.masks import make_identity

F32 = mybir.dt.float32
BF16 = mybir.dt.float16  # 16-bit compute dtype (f16: 10-bit mantissa)
U32 = mybir.dt.uint32
AF = mybir.ActivationFunctionType
ALU = mybir.AluOpType
AX = mybir.AxisListType

D, H, V, K = 300, 512, 8835, 50
L, MAX_LEN, ROUNDS = 20, 21, 10
BS = 160
NCORES = 8
PBS = [128, 32]
BOFF = [0, 128]
IL = 256
VP = 8960
NEG = -1.0e30
G3 = 3 * H
XROWS = [128, 128, D - 256]  # k-tile partition sizes for embeddings


def bcast_mid(ap, reps):
    return bass.AP(tensor=ap.tensor, offset=ap.offset,
                   ap=[ap.ap[0], [0, reps], ap.ap[1]])


def bcast_in(ap, reps):
    return bass.AP(tensor=ap.tensor, offset=ap.offset,
                   ap=[ap.ap[0], ap.ap[1], [0, reps]])


def build_nc():
    nc = bacc.Bacc()

    def din(name, shape, dt):
        return nc.dram_tensor(name, shape, dt, kind="ExternalInput")

    # decoder GRU weights (feature-major stationary tiles)
    w_gx = din("w_gx", [128, 3, G3], BF16)      # x side (D rows)
    w_gq = din("w_gq", [128, 4, G3], BF16)      # q-context side (H rows)
    w_gh = din("w_gh", [128, 4, G3], BF16)      # hidden side (n cols halved)
    w_at = din("w_at", [128, 2, G3], BF16)      # (Wic@ivW).T img-feat rows
    dbias = din("dbias", [128, 16], F32)
    # encoder GRU weights
    w_egx = din("w_egx", [128, 3, G3], BF16)
    w_egh = din("w_egh", [128, 4, G3], BF16)
    ebias = din("ebias", [128, 16], F32)
    # output projection
    w_out = din("w_out", [128, 8, D], BF16)     # h + qc rows
    w_a2t = din("w_a2t", [128, 2, D], BF16)     # (Wout_ic@ivW).T
    outb = din("outb", [1, D], BF16)            # + Wout_ic@iv_b
    # attention projections
    w_qk = din("w_qk", [128, 4, K], BF16)
    qkb = din("qkb", [1, K], BF16)
    w_qv = din("w_qv", [128, 4, H], BF16)
    qvb_c = din("qvb_c", [128, 4], F32)
    w_ak = din("w_ak", [128, 4, K], BF16)
    akb = din("akb", [1, K], BF16)
    w_ik = din("w_ik", [128, 2, K], BF16)
    ikb_c = din("ikb_c", [128, 1], F32)
    img_t = din("img_t", [128, 2, IL], BF16)
    # embeddings
    emb_bf = din("emb_bf", [V, D], BF16)
    emb_aug = din("emb_aug", [V, D + 1], F32)
    embt_bf = din("embt_bf", [128, 3, VP], BF16)
    # per-core indices and masks
    q_idx = din("q_idx", [128, 2 * L], U32)
    a_idx = din("a_idx", [128, 2 * L], U32)
    qe_mask = din("qe_mask", [128, 2, L], BF16)
    ie_mask = din("ie_mask", [128, 2, IL], BF16)

    out_o = nc.dram_tensor("out_o", [MAX_LEN, BS, D], F32, kind="ExternalOutput")

    with tile.TileContext(nc) as tc:
        with (
            tc.tile_pool(name="cw", bufs=1) as cw,
            tc.tile_pool(name="pers", bufs=1) as pers,
            tc.tile_pool(name="wk", bufs=2) as wk,
            tc.tile_pool(name="st", bufs=2) as st,
            tc.tile_pool(name="psz", bufs=4, space="PSUM") as psz,
            tc.tile_pool(name="psb", bufs=2, space="PSUM") as psb,
            tc.tile_pool(name="pss", bufs=2, space="PSUM") as pss,
        ):
            def load(pool, t, dt):
                s = pool.tile(list(t.shape), dt, name=t.name + "_sb")
                nc.sync.dma_start(s[:], t[:])
                return s

            s_ak = load(cw, w_ak, BF16)
            s_ik = load(cw, w_ik, BF16)
            s_imgt = load(cw, img_t, BF16)
            s_outb = load(cw, outb, BF16)
            s_akb = load(cw, akb, BF16)
            s_ikb = load(cw, ikb_c, F32)
            s_qvb = load(cw, qvb_c, F32)
            s_dbias = load(cw, dbias, F32)
            s_ebias = load(cw, ebias, F32)
            s_aidx = load(cw, a_idx, U32)
            s_qem = load(cw, qe_mask, BF16)
            s_iem = load(cw, ie_mask, BF16)
            # big persistent embedding-transpose table (used at step 19);
            # its DMA is issued inside the encoder so it doesn't delay startup
            s_embt = cw.tile([128, 3, VP], BF16, name="s_embt")

            ident_bf = cw.tile([128, 128], BF16)
            make_identity(nc, ident_bf[:])
            ones_bf = cw.tile([1, 128], BF16)
            nc.vector.memset(ones_bf[:], 1.0)
            sid4 = cw.tile([128, 32], BF16)
            for g4 in range(4):
                nc.vector.tensor_copy(sid4[32 * g4:32 * (g4 + 1), :],
                                      ident_bf[0:32, 0:32])
            iota8 = cw.tile([128, 8], F32)
            nc.gpsimd.iota(iota8[:], pattern=[[1, 8]], base=0, channel_multiplier=0,
                           allow_small_or_imprecise_dtypes=True)

            # persistent state (double-buffered hT)
            hT0 = pers.tile([128, 4, BS], BF16)
            hT1 = pers.tile([128, 4, BS], BF16)
            hTs = [hT0, hT1]
            h_f = pers.tile([128, 4, BS], F32)
            qk_b = pers.tile([128, 2, L, K], BF16)
            qv_b0 = pers.tile([128, L, H], BF16)
            qv_p1 = pers.tile([128, 5, H], BF16)
            ikt = pers.tile([128, IL], BF16)
            ptab = pers.tile([128, 2, G3], BF16)
            p2 = pers.tile([128, 2, D], BF16)
            a_bf = pers.tile([128, 2, K], BF16)
            qcT = pers.tile([128, 4, BS], BF16)
            iwT = pers.tile([128, 2, BS], BF16)
            dec20 = pers.tile([128, 3, BS], BF16)

            nc.vector.memset(hT0[:], 0.0)
            nc.vector.memset(h_f[:], 0.0)
            nc.vector.memset(qk_b[:], 0.0)
            nc.vector.memset(a_bf[:], 0.0)

            def tr(dst_sb_ap, src_sb_ap, pb, w, eng=None):
                pt = pss.tile([128, 128], BF16, tag="s", name="pt")
                nc.tensor.transpose(pt[:w, :pb], src_sb_ap, ident_bf[:pb, :pb])
                (eng or nc.vector).tensor_copy(dst_sb_ap, pt[:w, :pb])

            def fetch_x(idx_sb, t):
                xt = wk.tile([128, 3, BS], BF16, tag="xt", bufs=3, name="xt")
                for c, (pb, off) in enumerate(zip(PBS, BOFF)):
                    g = wk.tile([128, D], BF16, tag="gath", bufs=4, name="g")
                    nc.gpsimd.indirect_dma_start(
                        out=g[:pb], out_offset=None, in_=emb_bf[:],
                        in_offset=bass.IndirectOffsetOnAxis(
                            ap=idx_sb[:pb, 2 * t + c:2 * t + c + 1], axis=0))
                    for kt in range(3):
                        w = XROWS[kt]
                        tr(xt[:w, kt, off:off + pb], g[:pb, kt * 128:kt * 128 + w],
                           pb, w)
                return xt

            def emit_group(ps_ap, pairs):
                n = len(pairs)
                for i, (lh, rh) in enumerate(pairs):
                    nc.tensor.matmul(ps_ap, lh, rh, start=(i == 0), stop=(i == n - 1))

            # ---------------- one-time precompute ----------------
            with tc.tile_pool(name="pre", bufs=1) as pre:
                s_at = load(pre, w_at, BF16)
                s_a2t = load(pre, w_a2t, BF16)
                # ikt[K, IL] = image keys (transposed)
                psik = pss.tile([128, IL], F32, tag="s", name="psik")
                emit_group(psik[:K, :], [(s_ik[:, kt, :], s_imgt[:, kt, :])
                                         for kt in range(2)])
                nc.vector.tensor_scalar_add(ikt[:K, :], psik[:K, :], s_ikb[:K, :])

                # ptab[j, 1536] = (Wic@ivW @ img^T)^T tiles; p2[j, 300] likewise
                for jt in range(2):
                    for ch in range(3):
                        psp = pss.tile([128, 512], F32, tag="s", name="psp")
                        emit_group(psp[:, :],
                                   [(s_imgt[:, kt, jt * 128:(jt + 1) * 128],
                                     s_at[:, kt, ch * 512:(ch + 1) * 512])
                                    for kt in range(2)])
                        nc.scalar.copy(ptab[:, jt, ch * 512:(ch + 1) * 512],
                                       psp[:, :])
                    psp2 = pss.tile([128, 512], F32, tag="s", name="psp2")
                    emit_group(psp2[:, :D],
                               [(s_imgt[:, kt, jt * 128:(jt + 1) * 128],
                                 s_a2t[:, kt, :]) for kt in range(2)])
                    nc.scalar.copy(p2[:, jt, :], psp2[:, :D])

            # ---------------- feature-major GRU core ----------------
            def gru_fm(cur, nxt, gi_srcs, gh_w, bias_sb):
                """gi_srcs: list of (weight_sb, src_fn(kt)->AP, nkt, rows)
                covering x (+ qc/img for decoder); gh_w hidden weights with
                n-columns pre-halved; bias cols: [r0..3, z0..3, n0..3, bn0..3].
                Updates h_f in place and writes nxt (hT double buffer)."""
                rza = [psz.tile([128, 3, BS], F32, tag="rza", name="rza")
                       for _ in range(4)]
                # early members per chunk: hidden side ordered kt-major so the
                # PE can start as soon as each hT chunk of the previous step
                # lands, then x/img sides; late (attention-dependent) members
                # close the groups in gru_finish.
                pend = [[] for _ in range(4)]
                early = [[] for _ in range(4)]
                for kt in range(4):
                    for c in range(4):
                        for g in range(2):  # r, z hidden side
                            cs = slice(g * H + c * 128, g * H + (c + 1) * 128)
                            early[c].append((rza[c][:, g, :], gh_w[:, kt, cs],
                                             cur[:, kt, :]))
                for c in range(4):
                    for (wsb, srcf, nkt, rows, late) in gi_srcs:
                        for g in range(3):
                            cs = slice(g * H + c * 128, g * H + (c + 1) * 128)
                            for kt in range(nkt):
                                r = rows[kt]
                                mm = (rza[c][:, g, :], wsb[:r, kt, cs],
                                      srcf(kt, r))
                                (pend[c] if late else early[c]).append(mm)
                started = [False] * 4
                emit_seq = []
                for kt in range(4):
                    for c in range(4):
                        emit_seq.append((c, early[c][2 * kt]))
                        emit_seq.append((c, early[c][2 * kt + 1]))
                for c in range(4):
                    for mm in early[c][8:]:
                        emit_seq.append((c, mm))
                remaining = [len(early[c]) for c in range(4)]
                for c, (o, lh, rh) in emit_seq:
                    remaining[c] -= 1
                    nc.tensor.matmul(o, lh, rh, start=(not started[c]),
                                     stop=(not pend[c] and remaining[c] == 0))
                    started[c] = True
                # bn: hidden-side n gate (0.5-scaled weights); extract to SBUF
                # immediately (with bias) so the psum banks free up for qc
                bnbs = []
                for half in range(2):
                    bn = psb.tile([128, 2, BS], F32, tag="bq", name="bn")
                    pairs = []
                    for ci in range(2):
                        c = half * 2 + ci
                        cs = slice(2 * H + c * 128, 2 * H + (c + 1) * 128)
                        pairs += [(gh_w[:, kt, cs], cur[:, kt, :])
                                  for kt in range(4)]
                    for i, (lh, rh) in enumerate(pairs):
                        nc.tensor.matmul(bn[:, i // 4, :], lh, rh,
                                         start=(i == 0), stop=(i == len(pairs) - 1))
                    for ci in range(2):
                        c = half * 2 + ci
                        bnb = st.tile([128, BS], F32, tag="bnb", bufs=4,
                                      name="bnb")
                        nc.gpsimd.tensor_scalar_add(bnb[:], bn[:, ci, :],
                                                    bias_sb[:, 12 + c:13 + c])
                        bnbs.append(bnb)
                return rza, pend, bnbs

            def gru_finish(rza, pend, bnbs, nxt, bias_sb, mid_cb=None,
                           mid_at=None):
                # late members close each chunk's group; interleave chunks so
                # each bank finishes as late as its own last member. mid_cb is
                # emitted once index mid_at is reached (used at t=20 to place
                # the dec20 transposes between q-context and x members).
                npend = max(len(p) for p in pend) if pend else 0
                for i in range(npend):
                    if mid_cb is not None and i == mid_at:
                        mid_cb()
                        mid_cb = None
                    for c in range(4):
                        if i < len(pend[c]):
                            ps_ap, lh, rh = pend[c][i]
                            nc.tensor.matmul(ps_ap, lh, rh, start=False,
                                             stop=(i == len(pend[c]) - 1))
                for c in range(4):
                    ps = rza[c]
                    bnb = bnbs[c]
                    th_r = st.tile([128, BS], F32, tag="thr", name="th_r")
                    nc.scalar.activation(th_r[:], ps[:, 0, :], AF.Tanh,
                                         bias=bias_sb[:, c:c + 1], scale=0.5)
                    th_z = st.tile([128, BS], F32, tag="thz", name="th_z")
                    nc.scalar.activation(th_z[:], ps[:, 1, :], AF.Tanh,
                                         bias=bias_sb[:, 4 + c:5 + c], scale=0.5)
                    t1 = st.tile([128, BS], F32, tag="t1", name="t1")
                    nc.vector.scalar_tensor_tensor(t1[:], th_r[:], 1.0, bnb[:],
                                                   op0=ALU.add, op1=ALU.mult)
                    nc.vector.tensor_add(t1[:], t1[:], ps[:, 2, :])
                    n = st.tile([128, BS], F32, tag="n", name="n")
                    nc.scalar.activation(n[:], t1[:], AF.Tanh,
                                         bias=bias_sb[:, 8 + c:9 + c])
                    s1 = st.tile([128, BS], F32, tag="s1", name="s1")
                    nc.gpsimd.tensor_sub(s1[:], h_f[:, c, :], n[:])
                    u = st.tile([128, BS], F32, tag="u", name="u")
                    nc.vector.scalar_tensor_tensor(u[:], th_z[:], 1.0, s1[:],
                                                   op0=ALU.add, op1=ALU.mult)
                    nc.vector.scalar_tensor_tensor(h_f[:, c, :], u[:], 0.5, n[:],
                                                   op0=ALU.mult, op1=ALU.add)
                    nc.gpsimd.tensor_copy(nxt[:, c, :], h_f[:, c, :])

            # ---------------- encoder ----------------
            with tc.tile_pool(name="qp", bufs=1) as qp:
                s_egx = load(qp, w_egx, BF16)
                s_egh = load(qp, w_egh, BF16)
                s_qk = load(qp, w_qk, BF16)
                s_qv = load(qp, w_qv, BF16)
                s_qkb = load(qp, qkb, BF16)
                s_qidx = load(qp, q_idx, U32)
                for kt in range(3):
                    nc.sync.dma_start(s_embt[:, kt, :], embt_bf[:, kt, :])
                def qkv_proj(t, ht):
                    # qk / qv projections from h at encoder step t
                    for bt in range(2):
                        pb, off = PBS[bt], BOFF[bt]
                        sl = slice(off, off + pb)
                        psk = pss.tile([128, 512], F32, tag="s", name="psk")
                        pairs = [(ht[:, kt, sl], s_qk[:, kt, :]) for kt in range(4)]
                        pairs.append((ones_bf[:, :pb], s_qkb[:]))
                        emit_group(psk[:pb, :K], pairs)
                        nc.scalar.copy(qk_b[:pb, bt, t, :], psk[:pb, :K])
                        psv = pss.tile([128, 512], F32, tag="s", name="psv")
                        emit_group(psv[:pb, :],
                                   [(ht[:, kt, sl], s_qv[:, kt, :])
                                    for kt in range(4)])
                        if bt == 0:
                            nc.scalar.copy(qv_b0[:pb, t, :], psv[:pb, :])
                        else:
                            g4 = t % 4
                            nc.scalar.copy(
                                qv_p1[32 * g4:32 * (g4 + 1), t // 4, :],
                                psv[:pb, :])

                xt_n = fetch_x(s_qidx, 0)
                for t in range(L):
                    cur, nxt = hTs[t % 2], hTs[(t + 1) % 2]
                    xt = xt_n
                    srcs = [(s_egx, lambda kt, r, xt=xt: xt[:r, kt, :], 3,
                             XROWS, False)]
                    rza, pend, bns = gru_fm(cur, nxt, srcs, s_egh, s_ebias)
                    # previous step's projections fill the PE while this
                    # step's activation chain drains
                    if t > 0:
                        qkv_proj(t - 1, cur)
                    if t + 1 < L:
                        xt_n = fetch_x(s_qidx, t + 1)
                    gru_finish(rza, pend, bns, nxt, s_ebias)
                qkv_proj(L - 1, hTs[L % 2])

            nc.vector.memset(hT0[:], 0.0)
            nc.vector.memset(h_f[:], 0.0)

            # ---------------- decoder ----------------
            with tc.tile_pool(name="lg", bufs=1) as lg:
                s_gx = load(lg, w_gx, BF16)
                s_gq = load(lg, w_gq, BF16)
                s_gh = load(lg, w_gh, BF16)
                s_out = load(lg, w_out, BF16)
                o19T = lg.tile([128, 3, BS], BF16)
                nc.vector.memset(o19T[32:64, 2, :], 0.0)
                nc.vector.memset(o19T[64:65, 2, :], 1.0)
                o19_0 = lg.tile([128, D], F32)
                o19_1 = lg.tile([128, D], F32)
                o19_sb = [o19_0, o19_1]
                logit_sb = lg.tile([128, VP], BF16)

                def tail_logits(bt, second):
                    pb, off = PBS[bt], BOFF[bt]
                    for nci in range(18):
                        ncw = 512 if nci < 17 else V - 17 * 512
                        psl = pss.tile([128, 512], F32, tag="s", name="psl")
                        pairs = []
                        for kt in range(3):
                            nr = 128 if kt < 2 else 65
                            pairs.append((o19T[:nr, kt, off:off + pb],
                                          s_embt[:nr, kt,
                                                 nci * 512:nci * 512 + ncw]))
                        emit_group(psl[:pb, :ncw], pairs)
                        dst = logit_sb[:pb, nci * 512:nci * 512 + ncw]
                        m = nci % 3
                        if m == 1:
                            nc.gpsimd.tensor_copy(dst, psl[:pb, :ncw])
                        elif m == 2 and not second:
                            nc.vector.tensor_copy(dst, psl[:pb, :ncw])
                        else:
                            nc.scalar.copy(dst, psl[:pb, :ncw])

                def tail_scan(bt, first):
                    pb = PBS[bt]
                    if first:
                        nc.vector.memset(logit_sb[:, V:], -60000.0)
                    mx8 = st.tile([128, 8], BF16, tag="mx8", name="mx8")
                    nc.vector.max(mx8[:pb], logit_sb[:pb])
                    ix8 = st.tile([128, 8], U32, tag="ix8", name="ix8")
                    nc.vector.max_index(ix8[:pb], mx8[:pb], logit_sb[:pb])
                    return ix8

                def tail_rescore(bt, ix8):
                    """exact f32 rescoring of the top-8 (all on Pool) ->
                    gathered winner embedding"""
                    pb, off = PBS[bt], BOFF[bt]
                    g8 = wk.tile([128, 8, D + 1], F32, tag="gath8", bufs=1,
                                 name="g8")
                    nc.gpsimd.indirect_dma_start(
                        out=g8[:pb], out_offset=None, in_=emb_aug[:],
                        in_offset=bass.IndirectOffsetOnAxis(
                            ap=ix8[:pb, 0:8], axis=0))
                    scores = st.tile([128, 8], F32, tag="sco", name="scores")
                    for j in range(8):
                        pr = wk.tile([128, D], F32, tag="pr8", bufs=1,
                                     name="pr")
                        sj = st.tile([128, 1], F32, tag="sj", name="sj")
                        nc.gpsimd.scalar_tensor_tensor(
                            pr[:pb], o19_sb[bt][:pb], 1.0, g8[:pb, j, :D],
                            op0=ALU.mult, op1=ALU.mult, accum_out=sj[:pb])
                        nc.gpsimd.tensor_add(scores[:pb, j:j + 1],
                                             sj[:pb], g8[:pb, j, D:D + 1])
                    # argmax over the 8 rescored candidates (min index on tie)
                    m4 = st.tile([128, 4], F32, tag="m4", name="m4")
                    nc.gpsimd.tensor_max(m4[:pb], scores[:pb, 0:4],
                                         scores[:pb, 4:8])
                    nc.gpsimd.tensor_max(m4[:pb, 0:2], m4[:pb, 0:2],
                                         m4[:pb, 2:4])
                    mxs = st.tile([128, 1], F32, tag="mxs", name="mxs")
                    nc.gpsimd.tensor_max(mxs[:pb], m4[:pb, 0:1], m4[:pb, 1:2])
                    oh = st.tile([128, 8], F32, tag="oh", name="oh")
                    nc.gpsimd.tensor_scalar(out=oh[:pb], in0=scores[:pb],
                                            scalar1=mxs[:pb], scalar2=None,
                                            op0=ALU.is_equal)
                    ix8f = st.tile([128, 8], F32, tag="ix8f", name="ix8f")
                    nc.gpsimd.tensor_copy(ix8f[:pb], ix8[:pb])
                    nc.gpsimd.tensor_scalar_sub(ix8f[:pb], ix8f[:pb], 65536.0)
                    nc.gpsimd.tensor_mul(ix8f[:pb], oh[:pb], ix8f[:pb])
                    nc.gpsimd.tensor_scalar_add(ix8f[:pb], ix8f[:pb], 65536.0)
                    nc.gpsimd.tensor_tensor(ix8f[:pb, 0:4], ix8f[:pb, 0:4],
                                            ix8f[:pb, 4:8], op=ALU.min)
                    nc.gpsimd.tensor_tensor(ix8f[:pb, 0:2], ix8f[:pb, 0:2],
                                            ix8f[:pb, 2:4], op=ALU.min)
                    vsum = st.tile([128, 1], F32, tag="vsum", name="vsum")
                    nc.gpsimd.tensor_tensor(vsum[:pb], ix8f[:pb, 0:1],
                                            ix8f[:pb, 1:2], op=ALU.min)
                    vidx = st.tile([128, 1], U32, tag="vidx", name="vidx")
                    nc.gpsimd.tensor_copy(vidx[:pb], vsum[:pb])
                    gm = wk.tile([128, D], BF16, tag="gath", bufs=4, name="gm")
                    nc.gpsimd.indirect_dma_start(
                        out=gm[:pb], out_offset=None, in_=emb_bf[:],
                        in_offset=bass.IndirectOffsetOnAxis(
                            ap=vidx[:pb, 0:1], axis=0))
                    return gm

                tail_gms = {}

                def tail_emit():
                    tail_logits(1, second=False)
                    ix1 = tail_scan(1, first=True)
                    tail_logits(0, second=True)
                    ix0 = tail_scan(0, first=False)
                    tail_gms[1] = tail_rescore(1, ix1)
                    tail_gms[0] = tail_rescore(0, ix0)

                def tail_trs():
                    for bt in range(2):
                        pb, off = PBS[bt], BOFF[bt]
                        gm = tail_gms[bt]
                        for kt in range(3):
                            w = XROWS[kt]
                            tr(dec20[:w, kt, off:off + pb],
                               gm[:pb, kt * 128:kt * 128 + w], pb, w)

                xt_n = fetch_x(s_aidx, 0)
                for t in range(MAX_LEN):
                    cur, nxt = hTs[t % 2], hTs[(t + 1) % 2]
                    # --- attention query a = h@ak_W + ak_b ---
                    aT = st.tile([128, BS], BF16, tag="aT", name="aT")
                    for bt in range(2):
                        pb, off = PBS[bt], BOFF[bt]
                        sl = slice(off, off + pb)
                        psa = pss.tile([128, 512], F32, tag="s", name="psa")
                        pairs = [(cur[:, kt, sl], s_ak[:, kt, :]) for kt in range(4)]
                        pairs.append((ones_bf[:, :pb], s_akb[:]))
                        emit_group(psa[:pb, :K], pairs)
                        nc.scalar.copy(a_bf[:pb, bt, :], psa[:pb, :K])
                        tr(aT[:K, off:off + pb], a_bf[:pb, bt, :], pb, K)

                    # --- GRU early members (hidden + x sides) ---
                    xt = xt_n if t < L else dec20
                    late_x = (t == MAX_LEN - 1)
                    src_x = (s_gx, lambda kt, r, xt=xt: xt[:r, kt, :], 3,
                             XROWS, late_x)
                    srcs = [
                        (s_gq, lambda kt, r: qcT[:r, kt, :], 4, [128] * 4, True),
                        (ptab, lambda kt, r: iwT[:r, kt, :], 2, [128] * 2, True),
                    ]
                    srcs = srcs + [src_x] if late_x else [src_x] + srcs
                    rza, pend, bns = gru_fm(cur, nxt, srcs, s_gh, s_dbias)
                    if t + 1 < L:
                        xt_n = fetch_x(s_aidx, t + 1)

                    # --- question attention scores (DVE) ---
                    prod = wk.tile([128, 2, L, K], BF16, tag="prod", bufs=1,
                                   name="prod")
                    abc = bass.AP(tensor=a_bf.tensor, offset=a_bf[:, :, :].offset,
                                  ap=[a_bf[:, :, :].ap[0], a_bf[:, :, :].ap[1],
                                      [0, L], a_bf[:, :, :].ap[2]])
                    nc.vector.tensor_tensor(out=prod[:], in0=qk_b[:, :, :, :],
                                            in1=abc, op=ALU.mult)
                    qe = st.tile([128, 2, L], F32, tag="qe", name="qe")
                    nc.vector.tensor_reduce(qe[:], prod[:], axis=AX.X, op=ALU.add)
                    nc.vector.tensor_add(qe[:], qe[:], s_qem[:, :, :])
                    qw_bf = st.tile([128, 2, L], BF16, tag="qwb", name="qw_bf")
                    for bt in range(2):
                        pb = PBS[bt]
                        nm = st.tile([128, 1], F32, tag="nm", name="nm")
                        nc.vector.tensor_reduce(nm[:pb], qe[:pb, bt, :], axis=AX.X,
                                                op=ALU.max, negate=True)
                        ew = st.tile([128, L], F32, tag="ew", name="ew")
                        ssum = st.tile([128, 1], F32, tag="ssum", name="ssum")
                        nc.scalar.activation(ew[:pb], qe[:pb, bt, :], AF.Exp,
                                             bias=nm[:pb], scale=1.0,
                                             accum_out=ssum[:pb])
                        rs = st.tile([128, 1], F32, tag="rs", name="rs")
                        nc.vector.reciprocal(rs[:pb], ssum[:pb])
                        nc.vector.tensor_scalar_mul(qw_bf[:pb, bt, :], ew[:pb],
                                                    rs[:pb])

                    # --- image attention (needs aT) ---
                    for bt in range(2):
                        pb, off = PBS[bt], BOFF[bt]
                        psi = pss.tile([128, 512], F32, tag="s", name="psi")
                        nc.tensor.matmul(psi[:pb, :IL], aT[:K, off:off + pb],
                                         ikt[:K, :], start=True, stop=True)
                        iem = st.tile([128, IL], F32, tag="iem", name="iem")
                        nc.vector.tensor_add(iem[:pb], psi[:pb, :IL],
                                             s_iem[:pb, bt, :])
                        nmi = st.tile([128, 1], F32, tag="nmi", name="nmi")
                        nc.vector.tensor_reduce(nmi[:pb], iem[:pb], axis=AX.X,
                                                op=ALU.max, negate=True)
                        ewi = st.tile([128, IL], F32, tag="ewi", name="ewi")
                        ssi = st.tile([128, 1], F32, tag="ssi", name="ssi")
                        nc.scalar.activation(ewi[:pb], iem[:pb], AF.Exp,
                                             bias=nmi[:pb], scale=1.0,
                                             accum_out=ssi[:pb])
                        rsi = st.tile([128, 1], F32, tag="rsi", name="rsi")
                        nc.vector.reciprocal(rsi[:pb], ssi[:pb])
                        iwb = st.tile([128, IL], BF16, tag="iwb", name="iwb")
                        nc.vector.tensor_scalar_mul(iwb[:pb], ewi[:pb], rsi[:pb])
                        for c in range(2):
                            tr(iwT[:, c, off:off + pb],
                               iwb[:pb, c * 128:(c + 1) * 128], pb, 128,
                               eng=nc.gpsimd)

                    # --- question context, feature-major ---
                    dg = wk.tile([128, L, 128], BF16, tag="diag", bufs=1, name="dg")
                    hl = L // 2
                    ibh = ident_bf[:128, :128]
                    ident_h = bass.AP(tensor=ibh.tensor, offset=ibh.offset,
                                      ap=[ibh.ap[0], [0, hl], ibh.ap[1]])
                    nc.gpsimd.tensor_mul(dg[:, :hl, :],
                                         bcast_in(qw_bf[:, 0, :hl], 128), ident_h)
                    nc.vector.tensor_mul(dg[:, hl:, :],
                                         bcast_in(qw_bf[:, 0, hl:], 128), ident_h)
                    qw_pk = st.tile([128, 5], BF16, tag="qwpk", name="qw_pk")
                    for g4 in range(4):
                        nc.vector.tensor_copy(qw_pk[32 * g4:32 * (g4 + 1), :],
                                              qw_bf[0:32, 1, g4:L:4])
                    dg1 = wk.tile([128, 5, 32], BF16, tag="dg1", name="dg1")
                    sid_b = bass.AP(tensor=sid4.tensor, offset=sid4[:, :].offset,
                                    ap=[sid4[:, :].ap[0], [0, 5], sid4[:, :].ap[1]])
                    nc.vector.tensor_mul(dg1[:, :, :], bcast_in(qw_pk[:, :], 32),
                                         sid_b)
                    for half in range(2):
                        pq = psb.tile([128, 2, BS], F32, tag="bq", name="pq")
                        mms = []
                        for ci in range(2):
                            c = half * 2 + ci
                            cs = slice(c * 128, (c + 1) * 128)
                            for l in range(L):
                                mms.append((pq[:, ci, 0:128], qv_b0[:, l, cs],
                                            dg[:, l, :]))
                            for g in range(5):
                                mms.append((pq[:, ci, 128:160], qv_p1[:, g, cs],
                                            dg1[:, g, :]))
                        for i, (o, lh, rh) in enumerate(mms):
                            nc.tensor.matmul(o, lh, rh, start=(i == 0),
                                             stop=(i == len(mms) - 1))
                        for ci in range(2):
                            c = half * 2 + ci
                            nc.scalar.activation(qcT[:, c, :], pq[:, ci, :],
                                                 AF.Identity,
                                                 bias=s_qvb[:, c:c + 1])

                    # --- step-19 argmax tail (emitted before gru_finish of
                    # step 20 so its DVE scans overlap step-20 attention) ---
                    if t == MAX_LEN - 1:
                        tail_emit()
                        gru_finish(rza, pend, bns, nxt, s_dbias,
                                   mid_cb=tail_trs, mid_at=18)
                    else:
                        gru_finish(rza, pend, bns, nxt, s_dbias)

                    # --- output projection ---
                    for bt in range(2):
                        pb, off = PBS[bt], BOFF[bt]
                        sl = slice(off, off + pb)
                        pso = pss.tile([128, 512], F32, tag="s", name="pso")
                        pairs = [(nxt[:, k, sl], s_out[:, k, :]) for k in range(4)]
                        pairs += [(qcT[:, k, sl], s_out[:, 4 + k, :])
                                  for k in range(4)]
                        pairs += [(iwT[:, k, sl], p2[:, k, :]) for k in range(2)]
                        pairs.append((ones_bf[:, :pb], s_outb[:]))
                        emit_group(pso[:pb, :D], pairs)
                        osb = st.tile([128, D], F32, tag="osb", name="osb")
                        nc.scalar.copy(osb[:pb], pso[:pb, :D])
                        nc.sync.dma_start(out_o[t, off:off + pb, :], osb[:pb])
                        if t == MAX_LEN - 2:
                            nc.vector.tensor_copy(o19_sb[bt][:pb], osb[:pb])

                    # --- prep for the argmax tail: o19 transposed ---
                    if t == MAX_LEN - 2:
                        for bt in range(2):
                            pb, off = PBS[bt], BOFF[bt]
                            ob = st.tile([128, D], BF16, tag="ob", name="ob")
                            nc.scalar.copy(ob[:pb], o19_sb[bt][:pb])
                            for kt in range(3):
                                w = XROWS[kt]
                                tr(o19T[:w, kt, off:off + pb],
                                   ob[:pb, kt * 128:kt * 128 + w], pb, w)

    nc.compile()
    return nc


_NC_CACHE = None


def _get_nc():
    global _NC_CACHE
    if _NC_CACHE is None:
        _NC_CACHE = build_nc()
    return _NC_CACHE


def _pad_tiles(a, ntiles):
    rows, cols = a.shape
    out = np.zeros((128 * ntiles, cols), a.dtype)
    out[:rows] = a
    return np.ascontiguousarray(out.reshape(ntiles, 128, cols).transpose(1, 0, 2))


def _bias_cols(vec):
    """[1536] -> [128, 12] (col = gate*4 + chunk, partition = feature%128)"""
    return np.ascontiguousarray(vec.reshape(12, 128).T)


def _prep_shared(inputs):
    bf = np.float16
    f32 = np.float32
    eW = np.asarray(inputs["embed_W"], f32)
    d = {}
    wih = np.asarray(inputs["dec_W_ih"], f32)
    whh = np.asarray(inputs["dec_W_hh"], f32)
    bih = np.asarray(inputs["dec_b_ih"], f32)
    bhh = np.asarray(inputs["dec_b_hh"], f32)
    ivW = np.asarray(inputs["iv_W"], f32)
    ivb = np.asarray(inputs["iv_b"], f32)
    outW = np.asarray(inputs["out_W"], f32)
    outb_v = np.asarray(inputs["out_b"], f32)

    d["w_gx"] = _pad_tiles(wih[:, 0:D].T.astype(bf), 3)
    d["w_gq"] = _pad_tiles(wih[:, D:D + H].T.astype(bf), 4)
    ghT = whh.T.copy()
    ghT[:, 2 * H:] *= 0.5
    d["w_gh"] = _pad_tiles(ghT.astype(bf), 4)
    Wic = wih[:, D + H:]                      # [3H, H]
    A = Wic @ ivW                             # [3H, 256]
    d["w_at"] = _pad_tiles(np.ascontiguousarray(A.T).astype(bf), 2)
    icb = Wic @ ivb                           # folded img bias [3H]
    db = np.zeros((128, 16), f32)
    rzb = 0.5 * (bih + bhh + icb)
    db[:, 0:4] = rzb[0:H].reshape(4, 128).T
    db[:, 4:8] = rzb[H:2 * H].reshape(4, 128).T
    db[:, 8:12] = (bih + icb)[2 * H:].reshape(4, 128).T
    db[:, 12:16] = (0.5 * bhh[2 * H:]).reshape(4, 128).T
    d["dbias"] = db

    ewih = np.asarray(inputs["enc_W_ih"], f32)
    ewhh = np.asarray(inputs["enc_W_hh"], f32)
    ebih = np.asarray(inputs["enc_b_ih"], f32)
    ebhh = np.asarray(inputs["enc_b_hh"], f32)
    d["w_egx"] = _pad_tiles(ewih[:, 0:D].T.astype(bf), 3)
    eghT = ewhh.T.copy()
    eghT[:, 2 * H:] *= 0.5
    d["w_egh"] = _pad_tiles(eghT.astype(bf), 4)
    eb = np.zeros((128, 16), f32)
    erzb = 0.5 * (ebih + ebhh)
    eb[:, 0:4] = erzb[0:H].reshape(4, 128).T
    eb[:, 4:8] = erzb[H:2 * H].reshape(4, 128).T
    eb[:, 8:12] = ebih[2 * H:].reshape(4, 128).T
    eb[:, 12:16] = (0.5 * ebhh[2 * H:]).reshape(4, 128).T
    d["ebias"] = eb

    d["w_out"] = _pad_tiles(outW[:, 0:2 * H].T.astype(bf), 8)
    Woic = outW[:, 2 * H:]                    # [300, H]
    A2 = Woic @ ivW                           # [300, 256]
    d["w_a2t"] = _pad_tiles(np.ascontiguousarray(A2.T).astype(bf), 2)
    d["outb"] = np.ascontiguousarray(
        (outb_v + Woic @ ivb).astype(bf)[None, :])

    d["w_qk"] = _pad_tiles(np.asarray(inputs["qk_W"], f32).T.astype(bf), 4)
    d["qkb"] = np.ascontiguousarray(
        np.asarray(inputs["qk_b"], f32).astype(bf)[None, :])
    d["w_qv"] = _pad_tiles(np.asarray(inputs["qv_W"], f32).T.astype(bf), 4)
    d["qvb_c"] = np.ascontiguousarray(
        np.asarray(inputs["qv_b"], f32).reshape(4, 128).T)
    d["w_ak"] = _pad_tiles(np.asarray(inputs["ak_W"], f32).T.astype(bf), 4)
    d["akb"] = np.ascontiguousarray(
        np.asarray(inputs["ak_b"], f32).astype(bf)[None, :])
    d["w_ik"] = _pad_tiles(np.asarray(inputs["ik_W"], f32).T.astype(bf), 2)
    ikb = np.zeros((128, 1), f32)
    ikb[:K, 0] = np.asarray(inputs["ik_b"], f32)
    d["ikb_c"] = ikb
    d["emb_bf"] = eW.astype(bf)
    wd_b = np.asarray(inputs["wd_b"], f32)
    d["emb_aug"] = np.ascontiguousarray(np.concatenate([eW, wd_b[:, None]], 1))
    aug = np.zeros((128 * 3, VP), f32)
    aug[:D, :V] = eW.T
    aug[320, :V] = wd_b
    d["embt_bf"] = _pad_tiles(aug.astype(bf), 3)
    return d


def _idx_cols(seq_rows):
    out = np.zeros((128, 2 * L), np.uint32)
    for t in range(L):
        out[:, 2 * t] = seq_rows[0:128, t]
        out[:32, 2 * t + 1] = seq_rows[128:160, t]
    return out


def _build_maps(inputs, shared):
    f32 = np.float32
    bf = np.float16
    ques = np.asarray(inputs["ques_seqs"]).astype(np.uint32)
    ans = np.asarray(inputs["ans_seqs"]).astype(np.uint32)
    qlens = np.asarray(inputs["ques_lens"]).astype(np.int64)
    img = np.asarray(inputs["img_seqs"], f32)
    maps = []
    for s in range(NCORES):
        m = dict(shared)
        r0 = s * BS
        m["q_idx"] = _idx_cols(ques[r0:r0 + BS, :L])
        m["a_idx"] = _idx_cols(ans[r0:r0 + BS, :L])
        qm = np.full((128, 2, L), -60000.0, bf)
        lens = qlens[r0:r0 + BS]
        for bt, (pb, off) in enumerate(zip(PBS, BOFF)):
            for b in range(pb):
                qm[b, bt, :lens[off + b]] = 0.0
        m["qe_mask"] = qm
        im = np.full((128, 2, IL), -60000.0, bf)
        for bt, (pb, off) in enumerate(zip(PBS, BOFF)):
            for b in range(pb):
                gimg = (off + b) // ROUNDS
                im[b, bt, gimg * 16:(gimg + 1) * 16] = 0.0
        m["ie_mask"] = im
        imgs = img[s * 16:(s + 1) * 16].reshape(IL, 256)
        it = np.zeros((128 * 2, IL), f32)
        it[:256] = imgs.T
        m["img_t"] = np.ascontiguousarray(
            it.reshape(2, 128, IL).transpose(1, 0, 2)).astype(bf)
        maps.append(m)
    return maps


def kernel(**inputs):
    nc = _get_nc()
    shared = _prep_shared(inputs)
    in_maps = _build_maps(inputs, shared)
    from concourse.bass_utils import run_bass_kernel_spmd
    res = run_bass_kernel_spmd(nc, in_maps, core_ids=list(range(NCORES)))
    outs = []
    for s in range(NCORES):
        o = np.asarray(res.results[s]["out_o"])
        outs.append(np.ascontiguousarray(o.transpose(1, 0, 2)))
    return np.concatenate(outs, 0).astype(np.float32)


# revision 22
# speedup vs baseline: 1.0762x; 1.0762x over previous
"""Trainium2 Bass kernel for nn_BaselineAttnDecoder.

Data-parallel over 8 NeuronCores: each core handles 160 decode rows
(= 16 images x 10 rounds). All weights replicated.

Feature-major GRU: weights are the stationary matmul operand, the 160
batch rows stream as moving columns, so gates land directly in the
transposed layout the next step needs (no per-step h transposes).
Image attention is folded: P = (W_ih_ic @ iv_W) @ img^T and
P2 = (W_out_ic @ iv_W) @ img^T are computed once on device, so the
image context vector is never materialized; its bias contribution
(softmax weights sum to 1) is folded into gate/output biases on host.
Sigmoid is computed as 0.5 + 0.5*tanh(x/2) (with W_hh_n pre-halved) so
the whole kernel uses one activation table (exp_and_others).
The transposed embedding table for the step-19 logits is persisted in
SBUF (loaded during the encoder), with bf16 top-8 + exact f32 rescore
for the argmax re-embedding.
"""
import numpy as np
import ml_dtypes

import concourse.bass as bass
import concourse.bacc as bacc
import concourse.mybir as mybir
import concourse.tile as tile
from concourse.masks import make_identity

F32 = mybir.dt.float32
BF16 = mybir.dt.float16  # 16-bit compute dtype (f16: 10-bit mantissa)
U32 = mybir.dt.uint32
AF = mybir.ActivationFunctionType
ALU = mybir.AluOpType
AX = mybir.AxisListType

D, H, V, K = 300, 512, 8835, 50
L, MAX_LEN, ROUNDS = 20, 21, 10
BS = 160
NCORES = 8
PBS = [128, 32]
BOFF = [0, 128]
IL = 256
VP = 8960
NEG = -1.0e30
G3 = 3 * H
XROWS = [128, 128, D - 256]  # k-tile partition sizes for embeddings


def bcast_mid(ap, reps):
    return bass.AP(tensor=ap.tensor, offset=ap.offset,
                   ap=[ap.ap[0], [0, reps], ap.ap[1]])


def bcast_in(ap, reps):
    return bass.AP(tensor=ap.tensor, offset=ap.offset,
                   ap=[ap.ap[0], ap.ap[1], [0, reps]])


def build_nc():
    nc = bacc.Bacc()

    def din(name, shape, dt):
        return nc.dram_tensor(name, shape, dt, kind="ExternalInput")

    # decoder GRU weights (feature-major stationary tiles)
    w_gx = din("w_gx", [128, 3, G3], BF16)      # x side (D rows)
    w_gq = din("w_gq", [128, 4, G3], BF16)      # q-context side (H rows)
    w_gh = din("w_gh", [128, 4, G3], BF16)      # hidden side (n cols halved)
    w_at = din("w_at", [128, 2, G3], BF16)      # (Wic@ivW).T img-feat rows
    dbias = din("dbias", [128, 16], F32)
    # encoder GRU weights
    w_egx = din("w_egx", [128, 3, G3], BF16)
    w_egh = din("w_egh", [128, 4, G3], BF16)
    ebias = din("ebias", [128, 16], F32)
    # output projection
    w_out = din("w_out", [128, 8, D], BF16)     # h + qc rows
    w_a2t = din("w_a2t", [128, 2, D], BF16)     # (Wout_ic@ivW).T
    outb = din("outb", [1, D], BF16)            # + Wout_ic@iv_b
    # attention projections
    w_qk = din("w_qk", [128, 4, K], BF16)
    qkb = din("qkb", [1, K], BF16)
    w_qv = din("w_qv", [128, 4, H], BF16)
    qvb_c = din("qvb_c", [128, 4], F32)
    w_ak = din("w_ak", [128, 4, K], BF16)
    akb = din("akb", [1, K], BF16)
    w_ik = din("w_ik", [128, 2, K], BF16)
    ikb_c = din("ikb_c", [128, 1], F32)
    img_t = din("img_t", [128, 2, IL], BF16)
    # embeddings
    emb_bf = din("emb_bf", [V, D], BF16)
    emb_aug = din("emb_aug", [V, D + 1], F32)
    embt_bf = din("embt_bf", [128, 3, VP], BF16)
    # per-core indices and masks
    q_idx = din("q_idx", [128, 2 * L], U32)
    a_idx = din("a_idx", [128, 2 * L], U32)
    qe_mask = din("qe_mask", [128, 2, L], BF16)
    ie_mask = din("ie_mask", [128, 2, IL], BF16)

    out_o = nc.dram_tensor("out_o", [MAX_LEN, BS, D], F32, kind="ExternalOutput")

    with tile.TileContext(nc) as tc:
        with (
            tc.tile_pool(name="cw", bufs=1) as cw,
            tc.tile_pool(name="pers", bufs=1) as pers,
            tc.tile_pool(name="wk", bufs=2) as wk,
            tc.tile_pool(name="st", bufs=2) as st,
            tc.tile_pool(name="psz", bufs=4, space="PSUM") as psz,
            tc.tile_pool(name="psb", bufs=2, space="PSUM") as psb,
            tc.tile_pool(name="pss", bufs=2, space="PSUM") as pss,
        ):
            def load(pool, t, dt):
                s = pool.tile(list(t.shape), dt, name=t.name + "_sb")
                nc.sync.dma_start(s[:], t[:])
                return s

            s_ak = load(cw, w_ak, BF16)
            s_ik = load(cw, w_ik, BF16)
            s_outb = load(cw, outb, BF16)
            s_akb = load(cw, akb, BF16)
            s_ikb = load(cw, ikb_c, F32)
            s_qvb = load(cw, qvb_c, F32)
            s_dbias = load(cw, dbias, F32)
            s_ebias = load(cw, ebias, F32)
            s_aidx = load(cw, a_idx, U32)
            s_qem = load(cw, qe_mask, BF16)
            s_iem = load(cw, ie_mask, BF16)
            # big persistent embedding-transpose table (used at step 19);
            # its DMA is issued inside the encoder so it doesn't delay startup
            s_embt = cw.tile([128, 3, VP], BF16, name="s_embt")

            ident_bf = cw.tile([128, 128], BF16)
            make_identity(nc, ident_bf[:])
            ones_bf = cw.tile([1, 128], BF16)
            nc.vector.memset(ones_bf[:], 1.0)
            sid4 = cw.tile([128, 32], BF16)
            for g4 in range(4):
                nc.vector.tensor_copy(sid4[32 * g4:32 * (g4 + 1), :],
                                      ident_bf[0:32, 0:32])
            iota8 = cw.tile([128, 8], F32)
            nc.gpsimd.iota(iota8[:], pattern=[[1, 8]], base=0, channel_multiplier=0,
                           allow_small_or_imprecise_dtypes=True)

            # persistent state (double-buffered hT)
            hT0 = pers.tile([128, 4, BS], BF16)
            hT1 = pers.tile([128, 4, BS], BF16)
            hTs = [hT0, hT1]
            h_f = pers.tile([128, 4, BS], F32)
            qk_b = pers.tile([128, 2, L, K], BF16)
            qv_b0 = pers.tile([128, L, H], BF16)
            qv_p1 = pers.tile([128, 5, H], BF16)
            ikt = pers.tile([128, IL], BF16)
            ptab = pers.tile([128, 2, G3], BF16)
            p2 = pers.tile([128, 2, D], BF16)
            a_bf = pers.tile([128, 2, K], BF16)
            qcT = pers.tile([128, 4, BS], BF16)
            iwT = pers.tile([128, 2, BS], BF16)
            dec20 = pers.tile([128, 3, BS], BF16)

            nc.vector.memset(hT0[:], 0.0)
            nc.vector.memset(h_f[:], 0.0)
            nc.vector.memset(qk_b[:], 0.0)
            nc.vector.memset(a_bf[:], 0.0)

            def tr(dst_sb_ap, src_sb_ap, pb, w, eng=None):
                pt = pss.tile([128, 128], BF16, tag="s", name="pt")
                nc.tensor.transpose(pt[:w, :pb], src_sb_ap, ident_bf[:pb, :pb])
                (eng or nc.vector).tensor_copy(dst_sb_ap, pt[:w, :pb])

            def fetch_x(idx_sb, t):
                xt = wk.tile([128, 3, BS], BF16, tag="xt", bufs=3, name="xt")
                for c, (pb, off) in enumerate(zip(PBS, BOFF)):
                    g = wk.tile([128, D], BF16, tag="gath", bufs=4, name="g")
                    nc.gpsimd.indirect_dma_start(
                        out=g[:pb], out_offset=None, in_=emb_bf[:],
                        in_offset=bass.IndirectOffsetOnAxis(
                            ap=idx_sb[:pb, 2 * t + c:2 * t + c + 1], axis=0))
                    for kt in range(3):
                        w = XROWS[kt]
                        tr(xt[:w, kt, off:off + pb], g[:pb, kt * 128:kt * 128 + w],
                           pb, w)
                return xt

            def emit_group(ps_ap, pairs):
                n = len(pairs)
                for i, (lh, rh) in enumerate(pairs):
                    nc.tensor.matmul(ps_ap, lh, rh, start=(i == 0), stop=(i == n - 1))

            # ---------------- one-time precompute ----------------
            with tc.tile_pool(name="pre", bufs=1) as pre:
                s_imgt = load(pre, img_t, BF16)
                s_at = load(pre, w_at, BF16)
                s_a2t = load(pre, w_a2t, BF16)
                # ikt[K, IL] = image keys (transposed)
                psik = pss.tile([128, IL], F32, tag="s", name="psik")
                emit_group(psik[:K, :], [(s_ik[:, kt, :], s_imgt[:, kt, :])
                                         for kt in range(2)])
                nc.vector.tensor_scalar_add(ikt[:K, :], psik[:K, :], s_ikb[:K, :])

                # ptab[j, 1536] = (Wic@ivW @ img^T)^T tiles; p2[j, 300] likewise
                for jt in range(2):
                    for ch in range(3):
                        psp = pss.tile([128, 512], F32, tag="s", name="psp")
                        emit_group(psp[:, :],
                                   [(s_imgt[:, kt, jt * 128:(jt + 1) * 128],
                                     s_at[:, kt, ch * 512:(ch + 1) * 512])
                                    for kt in range(2)])
                        nc.scalar.copy(ptab[:, jt, ch * 512:(ch + 1) * 512],
                                       psp[:, :])
                    psp2 = pss.tile([128, 512], F32, tag="s", name="psp2")
                    emit_group(psp2[:, :D],
                               [(s_imgt[:, kt, jt * 128:(jt + 1) * 128],
                                 s_a2t[:, kt, :]) for kt in range(2)])
                    nc.scalar.copy(p2[:, jt, :], psp2[:, :D])

            # ---------------- feature-major GRU core ----------------
            def gru_fm(cur, nxt, gi_srcs, gh_w, bias_sb):
                """gi_srcs: list of (weight_sb, src_fn(kt)->AP, nkt, rows)
                covering x (+ qc/img for decoder); gh_w hidden weights with
                n-columns pre-halved; bias cols: [r0..3, z0..3, n0..3, bn0..3].
                Updates h_f in place and writes nxt (hT double buffer)."""
                rza = [psz.tile([128, 3, BS], F32, tag="rza", name="rza")
                       for _ in range(4)]
                # early members per chunk: hidden side ordered kt-major so the
                # PE can start as soon as each hT chunk of the previous step
                # lands, then x/img sides; late (attention-dependent) members
                # close the groups in gru_finish.
                pend = [[] for _ in range(4)]
                early = [[] for _ in range(4)]
                for kt in range(4):
                    for c in range(4):
                        for g in range(2):  # r, z hidden side
                            cs = slice(g * H + c * 128, g * H + (c + 1) * 128)
                            early[c].append((rza[c][:, g, :], gh_w[:, kt, cs],
                                             cur[:, kt, :]))
                for c in range(4):
                    for (wsb, srcf, nkt, rows, late) in gi_srcs:
                        for g in range(3):
                            cs = slice(g * H + c * 128, g * H + (c + 1) * 128)
                            for kt in range(nkt):
                                r = rows[kt]
                                mm = (rza[c][:, g, :], wsb[:r, kt, cs],
                                      srcf(kt, r))
                                (pend[c] if late else early[c]).append(mm)
                started = [False] * 4
                emit_seq = []
                for kt in range(4):
                    for c in range(4):
                        emit_seq.append((c, early[c][2 * kt]))
                        emit_seq.append((c, early[c][2 * kt + 1]))
                for c in range(4):
                    for mm in early[c][8:]:
                        emit_seq.append((c, mm))
                remaining = [len(early[c]) for c in range(4)]
                for c, (o, lh, rh) in emit_seq:
                    remaining[c] -= 1
                    nc.tensor.matmul(o, lh, rh, start=(not started[c]),
                                     stop=(not pend[c] and remaining[c] == 0))
                    started[c] = True
                # bn: hidden-side n gate (0.5-scaled weights); extract to SBUF
                # immediately (with bias) so the psum banks free up for qc
                bnbs = []
                for half in range(2):
                    bn = psb.tile([128, 2, BS], F32, tag="bq", name="bn")
                    pairs = []
                    for ci in range(2):
                        c = half * 2 + ci
                        cs = slice(2 * H + c * 128, 2 * H + (c + 1) * 128)
                        pairs += [(gh_w[:, kt, cs], cur[:, kt, :])
                                  for kt in range(4)]
                    for i, (lh, rh) in enumerate(pairs):
                        nc.tensor.matmul(bn[:, i // 4, :], lh, rh,
                                         start=(i == 0), stop=(i == len(pairs) - 1))
                    for ci in range(2):
                        c = half * 2 + ci
                        bnb = st.tile([128, BS], F32, tag="bnb", bufs=4,
                                      name="bnb")
                        nc.gpsimd.tensor_scalar_add(bnb[:], bn[:, ci, :],
                                                    bias_sb[:, 12 + c:13 + c])
                        bnbs.append(bnb)
                return rza, pend, bnbs

            def gru_finish(rza, pend, bnbs, nxt, bias_sb, mid_cb=None,
                           mid_at=None):
                # late members close each chunk's group, chunk-major so chunk
                # 0's activations overlap chunk 1's matmuls. mid_cb is emitted
                # between member index mid_at and the rest (used at t=20 to
                # place the dec20 transposes before the x members).
                for c in range(4):
                    for i, (ps_ap, lh, rh) in enumerate(pend[c][:mid_at]):
                        nc.tensor.matmul(ps_ap, lh, rh, start=False,
                                         stop=(i == len(pend[c]) - 1))
                if mid_cb is not None:
                    mid_cb()
                if mid_at is not None:
                    for c in range(4):
                        for j, (ps_ap, lh, rh) in enumerate(pend[c][mid_at:]):
                            i = mid_at + j
                            nc.tensor.matmul(ps_ap, lh, rh, start=False,
                                             stop=(i == len(pend[c]) - 1))
                for c in range(4):
                    ps = rza[c]
                    bnb = bnbs[c]
                    th_r = st.tile([128, BS], F32, tag="thr", name="th_r")
                    nc.scalar.activation(th_r[:], ps[:, 0, :], AF.Tanh,
                                         bias=bias_sb[:, c:c + 1], scale=0.5)
                    th_z = st.tile([128, BS], F32, tag="thz", name="th_z")
                    nc.scalar.activation(th_z[:], ps[:, 1, :], AF.Tanh,
                                         bias=bias_sb[:, 4 + c:5 + c], scale=0.5)
                    t1 = st.tile([128, BS], F32, tag="t1", name="t1")
                    nc.vector.scalar_tensor_tensor(t1[:], th_r[:], 1.0, bnb[:],
                                                   op0=ALU.add, op1=ALU.mult)
                    nc.vector.tensor_add(t1[:], t1[:], ps[:, 2, :])
                    n = st.tile([128, BS], F32, tag="n", name="n")
                    nc.scalar.activation(n[:], t1[:], AF.Tanh,
                                         bias=bias_sb[:, 8 + c:9 + c])
                    s1 = st.tile([128, BS], F32, tag="s1", name="s1")
                    nc.gpsimd.tensor_sub(s1[:], h_f[:, c, :], n[:])
                    u = st.tile([128, BS], F32, tag="u", name="u")
                    nc.vector.scalar_tensor_tensor(u[:], th_z[:], 1.0, s1[:],
                                                   op0=ALU.add, op1=ALU.mult)
                    nc.vector.scalar_tensor_tensor(h_f[:, c, :], u[:], 0.5, n[:],
                                                   op0=ALU.mult, op1=ALU.add)
                    nc.gpsimd.tensor_copy(nxt[:, c, :], h_f[:, c, :])

            # ---------------- encoder ----------------
            with tc.tile_pool(name="qp", bufs=1) as qp:
                s_egx = load(qp, w_egx, BF16)
                s_egh = load(qp, w_egh, BF16)
                s_qk = load(qp, w_qk, BF16)
                s_qv = load(qp, w_qv, BF16)
                s_qkb = load(qp, qkb, BF16)
                s_qidx = load(qp, q_idx, U32)
                for kt in range(3):
                    nc.sync.dma_start(s_embt[:, kt, :], embt_bf[:, kt, :])
                def qkv_proj(t, ht):
                    # qk / qv projections from h at encoder step t
                    for bt in range(2):
                        pb, off = PBS[bt], BOFF[bt]
                        sl = slice(off, off + pb)
                        psk = pss.tile([128, 512], F32, tag="s", name="psk")
                        pairs = [(ht[:, kt, sl], s_qk[:, kt, :]) for kt in range(4)]
                        pairs.append((ones_bf[:, :pb], s_qkb[:]))
                        emit_group(psk[:pb, :K], pairs)
                        nc.scalar.copy(qk_b[:pb, bt, t, :], psk[:pb, :K])
                        psv = pss.tile([128, 512], F32, tag="s", name="psv")
                        emit_group(psv[:pb, :],
                                   [(ht[:, kt, sl], s_qv[:, kt, :])
                                    for kt in range(4)])
                        if bt == 0:
                            nc.scalar.copy(qv_b0[:pb, t, :], psv[:pb, :])
                        else:
                            g4 = t % 4
                            nc.scalar.copy(
                                qv_p1[32 * g4:32 * (g4 + 1), t // 4, :],
                                psv[:pb, :])

                xt_n = fetch_x(s_qidx, 0)
                for t in range(L):
                    cur, nxt = hTs[t % 2], hTs[(t + 1) % 2]
                    xt = xt_n
                    srcs = [(s_egx, lambda kt, r, xt=xt: xt[:r, kt, :], 3,
                             XROWS, False)]
                    rza, pend, bns = gru_fm(cur, nxt, srcs, s_egh, s_ebias)
                    # previous step's projections fill the PE while this
                    # step's activation chain drains
                    if t > 0:
                        qkv_proj(t - 1, cur)
                    if t + 1 < L:
                        xt_n = fetch_x(s_qidx, t + 1)
                    gru_finish(rza, pend, bns, nxt, s_ebias)
                qkv_proj(L - 1, hTs[L % 2])

            nc.vector.memset(hT0[:], 0.0)
            nc.vector.memset(h_f[:], 0.0)

            # ---------------- decoder ----------------
            with tc.tile_pool(name="lg", bufs=1) as lg:
                s_gx = load(lg, w_gx, BF16)
                s_gq = load(lg, w_gq, BF16)
                s_gh = load(lg, w_gh, BF16)
                s_out = load(lg, w_out, BF16)
                o19T = lg.tile([128, 3, BS], BF16)
                nc.vector.memset(o19T[32:64, 2, :], 0.0)
                nc.vector.memset(o19T[64:65, 2, :], 1.0)
                o19_0 = lg.tile([128, D], F32)
                o19_1 = lg.tile([128, D], F32)
                o19_sb = [o19_0, o19_1]
                RW = 18 * 256               # pairwise-reduced scan width
                NPAIR = (V + 1) // 2        # 4418 valid pairs
                red0 = lg.tile([128, RW], BF16)
                red1 = lg.tile([128, RW], BF16)
                reds = [red0, red1]

                def tail_logits(bt, second):
                    """logit blocks on PE; pairwise max straight out of PSUM
                    into the reduced scan buffer (no full logit storage)."""
                    pb, off = PBS[bt], BOFF[bt]
                    red = reds[bt]
                    nc.vector.memset(red[:, 2 * NPAIR // 2:], -60000.0)
                    for nci in range(18):
                        ncw = 512 if nci < 17 else V - 17 * 512
                        psl = pss.tile([128, 512], F32, tag="s", name="psl")
                        pairs = []
                        for kt in range(3):
                            nr = 128 if kt < 2 else 65
                            pairs.append((o19T[:nr, kt, off:off + pb],
                                          s_embt[:nr, kt,
                                                 nci * 512:nci * 512 + ncw]))
                        emit_group(psl[:pb, :ncw], pairs)
                        ro = nci * 256
                        npr = ncw // 2
                        eng = nc.gpsimd if nci % 3 == 1 else nc.vector
                        eng.tensor_tensor(red[:pb, ro:ro + npr],
                                          psl[:pb, 0:2 * npr:2],
                                          psl[:pb, 1:2 * npr:2], op=ALU.max)
                        if ncw % 2:
                            nc.vector.tensor_copy(
                                red[:pb, ro + npr:ro + npr + 1],
                                psl[:pb, ncw - 1:ncw])

                def tail_scan(bt, first):
                    pb = PBS[bt]
                    red = reds[bt]
                    mx8 = st.tile([128, 8], BF16, tag="mx8", name="mx8")
                    nc.vector.max(mx8[:pb], red[:pb])
                    ix8 = st.tile([128, 8], U32, tag="ix8", name="ix8")
                    nc.vector.max_index(ix8[:pb], mx8[:pb], red[:pb])
                    return ix8

                def tail_rescore(bt, ix8):
                    """each top-8 pair -> 2 vocab candidates; exact f32
                    rescore of all 16 (wave A on Pool, wave B on DVE)."""
                    pb, off = PBS[bt], BOFF[bt]
                    cand = st.tile([128, 16], F32, tag="cand", name="cand")
                    nc.gpsimd.tensor_copy(cand[:pb, 0:8], ix8[:pb])
                    nc.gpsimd.tensor_scalar_mul(cand[:pb, 0:8], cand[:pb, 0:8],
                                                2.0)
                    nc.gpsimd.tensor_scalar_add(cand[:pb, 8:16],
                                                cand[:pb, 0:8], 1.0)
                    nc.gpsimd.tensor_scalar_min(cand[:pb], cand[:pb],
                                                float(V - 1))
                    cand_u = st.tile([128, 16], U32, tag="candu", name="cand_u")
                    nc.gpsimd.tensor_copy(cand_u[:pb], cand[:pb])
                    scores = st.tile([128, 16], F32, tag="sco", name="scores")
                    g8 = wk.tile([128, 8, D + 1], F32, tag="gath8", bufs=1,
                                 name="g8")
                    nc.gpsimd.indirect_dma_start(
                        out=g8[:pb], out_offset=None, in_=emb_aug[:],
                        in_offset=bass.IndirectOffsetOnAxis(
                            ap=cand_u[:pb, 0:8], axis=0))
                    for j in range(8):
                        pr = wk.tile([128, D], F32, tag="pr8", bufs=2,
                                     name="pr")
                        sj = st.tile([128, 1], F32, tag="sj", name="sj")
                        nc.gpsimd.scalar_tensor_tensor(
                            pr[:pb], o19_sb[bt][:pb], 1.0, g8[:pb, j, :D],
                            op0=ALU.mult, op1=ALU.mult, accum_out=sj[:pb])
                        nc.gpsimd.tensor_add(scores[:pb, j:j + 1],
                                             sj[:pb], g8[:pb, j, D:D + 1])
                    g8b = wk.tile([128, 8, D + 1], F32, tag="gath8", bufs=1,
                                  name="g8b")
                    nc.gpsimd.indirect_dma_start(
                        out=g8b[:pb], out_offset=None, in_=emb_aug[:],
                        in_offset=bass.IndirectOffsetOnAxis(
                            ap=cand_u[:pb, 8:16], axis=0))
                    for j in range(8):
                        prb = wk.tile([128, D], F32, tag="pr8", bufs=2,
                                      name="prb")
                        sjb = st.tile([128, 1], F32, tag="sjb", name="sjb")
                        nc.vector.tensor_tensor_reduce(
                            out=prb[:pb], in0=o19_sb[bt][:pb],
                            in1=g8b[:pb, j, :D], scale=1.0, scalar=0.0,
                            op0=ALU.mult, op1=ALU.add, accum_out=sjb[:pb])
                        nc.vector.tensor_add(scores[:pb, 8 + j:9 + j],
                                             sjb[:pb], g8b[:pb, j, D:D + 1])
                    # argmax over the 16 rescored candidates (min idx on tie)
                    m8 = st.tile([128, 8], F32, tag="m8", name="m8")
                    nc.gpsimd.tensor_max(m8[:pb], scores[:pb, 0:8],
                                         scores[:pb, 8:16])
                    nc.gpsimd.tensor_max(m8[:pb, 0:4], m8[:pb, 0:4],
                                         m8[:pb, 4:8])
                    nc.gpsimd.tensor_max(m8[:pb, 0:2], m8[:pb, 0:2],
                                         m8[:pb, 2:4])
                    mxs = st.tile([128, 1], F32, tag="mxs", name="mxs")
                    nc.gpsimd.tensor_max(mxs[:pb], m8[:pb, 0:1], m8[:pb, 1:2])
                    oh = st.tile([128, 16], F32, tag="oh", name="oh")
                    nc.gpsimd.tensor_scalar(out=oh[:pb], in0=scores[:pb],
                                            scalar1=mxs[:pb], scalar2=None,
                                            op0=ALU.is_equal)
                    sel = st.tile([128, 16], F32, tag="sel", name="sel")
                    nc.gpsimd.tensor_scalar_sub(sel[:pb], cand[:pb], 65536.0)
                    nc.gpsimd.tensor_mul(sel[:pb], oh[:pb], sel[:pb])
                    nc.gpsimd.tensor_scalar_add(sel[:pb], sel[:pb], 65536.0)
                    nc.gpsimd.tensor_tensor(sel[:pb, 0:8], sel[:pb, 0:8],
                                            sel[:pb, 8:16], op=ALU.min)
                    nc.gpsimd.tensor_tensor(sel[:pb, 0:4], sel[:pb, 0:4],
                                            sel[:pb, 4:8], op=ALU.min)
                    nc.gpsimd.tensor_tensor(sel[:pb, 0:2], sel[:pb, 0:2],
                                            sel[:pb, 2:4], op=ALU.min)
                    vsum = st.tile([128, 1], F32, tag="vsum", name="vsum")
                    nc.gpsimd.tensor_tensor(vsum[:pb], sel[:pb, 0:1],
                                            sel[:pb, 1:2], op=ALU.min)
                    vidx = st.tile([128, 1], U32, tag="vidx", name="vidx")
                    nc.gpsimd.tensor_copy(vidx[:pb], vsum[:pb])
                    gm = wk.tile([128, D], BF16, tag="gath", bufs=4, name="gm")
                    nc.gpsimd.indirect_dma_start(
                        out=gm[:pb], out_offset=None, in_=emb_bf[:],
                        in_offset=bass.IndirectOffsetOnAxis(
                            ap=vidx[:pb, 0:1], axis=0))
                    return gm

                tail_gms = {}

                def tail_emit():
                    tail_logits(1, second=False)
                    ix1 = tail_scan(1, first=True)
                    tail_logits(0, second=True)
                    ix0 = tail_scan(0, first=False)
                    tail_gms[1] = tail_rescore(1, ix1)
                    tail_gms[0] = tail_rescore(0, ix0)

                def tail_trs():
                    for bt in range(2):
                        pb, off = PBS[bt], BOFF[bt]
                        gm = tail_gms[bt]
                        for kt in range(3):
                            w = XROWS[kt]
                            tr(dec20[:w, kt, off:off + pb],
                               gm[:pb, kt * 128:kt * 128 + w], pb, w)

                xt_n = fetch_x(s_aidx, 0)
                for t in range(MAX_LEN):
                    cur, nxt = hTs[t % 2], hTs[(t + 1) % 2]
                    # --- attention query a = h@ak_W + ak_b ---
                    aT = st.tile([128, BS], BF16, tag="aT", name="aT")
                    for bt in range(2):
                        pb, off = PBS[bt], BOFF[bt]
                        sl = slice(off, off + pb)
                        psa = pss.tile([128, 512], F32, tag="s", name="psa")
                        pairs = [(cur[:, kt, sl], s_ak[:, kt, :]) for kt in range(4)]
                        pairs.append((ones_bf[:, :pb], s_akb[:]))
                        emit_group(psa[:pb, :K], pairs)
                        nc.scalar.copy(a_bf[:pb, bt, :], psa[:pb, :K])
                        tr(aT[:K, off:off + pb], a_bf[:pb, bt, :], pb, K)

                    # --- GRU early members (hidden + x sides) ---
                    xt = xt_n if t < L else dec20
                    late_x = (t == MAX_LEN - 1)
                    src_x = (s_gx, lambda kt, r, xt=xt: xt[:r, kt, :], 3,
                             XROWS, late_x)
                    srcs = [
                        (s_gq, lambda kt, r: qcT[:r, kt, :], 4, [128] * 4, True),
                        (ptab, lambda kt, r: iwT[:r, kt, :], 2, [128] * 2, True),
                    ]
                    srcs = srcs + [src_x] if late_x else [src_x] + srcs
                    rza, pend, bns = gru_fm(cur, nxt, srcs, s_gh, s_dbias)
                    if t + 1 < L:
                        xt_n = fetch_x(s_aidx, t + 1)

                    # --- question attention scores (DVE) ---
                    prod = wk.tile([128, 2, L, K], BF16, tag="prod", bufs=1,
                                   name="prod")
                    abc = bass.AP(tensor=a_bf.tensor, offset=a_bf[:, :, :].offset,
                                  ap=[a_bf[:, :, :].ap[0], a_bf[:, :, :].ap[1],
                                      [0, L], a_bf[:, :, :].ap[2]])
                    nc.vector.tensor_tensor(out=prod[:], in0=qk_b[:, :, :, :],
                                            in1=abc, op=ALU.mult)
                    qe = st.tile([128, 2, L], F32, tag="qe", name="qe")
                    nc.vector.tensor_reduce(qe[:], prod[:], axis=AX.X, op=ALU.add)
                    nc.vector.tensor_add(qe[:], qe[:], s_qem[:, :, :])
                    qw_bf = st.tile([128, 2, L], BF16, tag="qwb", name="qw_bf")
                    for bt in range(2):
                        pb = PBS[bt]
                        nm = st.tile([128, 1], F32, tag="nm", name="nm")
                        nc.vector.tensor_reduce(nm[:pb], qe[:pb, bt, :], axis=AX.X,
                                                op=ALU.max, negate=True)
                        ew = st.tile([128, L], F32, tag="ew", name="ew")
                        ssum = st.tile([128, 1], F32, tag="ssum", name="ssum")
                        nc.scalar.activation(ew[:pb], qe[:pb, bt, :], AF.Exp,
                                             bias=nm[:pb], scale=1.0,
                                             accum_out=ssum[:pb])
                        rs = st.tile([128, 1], F32, tag="rs", name="rs")
                        nc.vector.reciprocal(rs[:pb], ssum[:pb])
                        nc.vector.tensor_scalar_mul(qw_bf[:pb, bt, :], ew[:pb],
                                                    rs[:pb])

                    # --- image attention (needs aT) ---
                    for bt in range(2):
                        pb, off = PBS[bt], BOFF[bt]
                        psi = pss.tile([128, 512], F32, tag="s", name="psi")
                        nc.tensor.matmul(psi[:pb, :IL], aT[:K, off:off + pb],
                                         ikt[:K, :], start=True, stop=True)
                        iem = st.tile([128, IL], F32, tag="iem", name="iem")
                        nc.vector.tensor_add(iem[:pb], psi[:pb, :IL],
                                             s_iem[:pb, bt, :])
                        nmi = st.tile([128, 1], F32, tag="nmi", name="nmi")
                        nc.vector.tensor_reduce(nmi[:pb], iem[:pb], axis=AX.X,
                                                op=ALU.max, negate=True)
                        ewi = st.tile([128, IL], F32, tag="ewi", name="ewi")
                        ssi = st.tile([128, 1], F32, tag="ssi", name="ssi")
                        nc.scalar.activation(ewi[:pb], iem[:pb], AF.Exp,
                                             bias=nmi[:pb], scale=1.0,
                                             accum_out=ssi[:pb])
                        rsi = st.tile([128, 1], F32, tag="rsi", name="rsi")
                        nc.vector.reciprocal(rsi[:pb], ssi[:pb])
                        iwb = st.tile([128, IL], BF16, tag="iwb", name="iwb")
                        nc.vector.tensor_scalar_mul(iwb[:pb], ewi[:pb], rsi[:pb])
                        for c in range(2):
                            tr(iwT[:, c, off:off + pb],
                               iwb[:pb, c * 128:(c + 1) * 128], pb, 128,
                               eng=nc.gpsimd)

                    # --- question context, feature-major ---
                    dg = wk.tile([128, L, 128], BF16, tag="diag", bufs=1, name="dg")
                    hl = L // 2
                    ibh = ident_bf[:128, :128]
                    ident_h = bass.AP(tensor=ibh.tensor, offset=ibh.offset,
                                      ap=[ibh.ap[0], [0, hl], ibh.ap[1]])
                    nc.gpsimd.tensor_mul(dg[:, :hl, :],
                                         bcast_in(qw_bf[:, 0, :hl], 128), ident_h)
                    nc.vector.tensor_mul(dg[:, hl:, :],
                                         bcast_in(qw_bf[:, 0, hl:], 128), ident_h)
                    qw_pk = st.tile([128, 5], BF16, tag="qwpk", name="qw_pk")
                    for g4 in range(4):
                        nc.vector.tensor_copy(qw_pk[32 * g4:32 * (g4 + 1), :],
                                              qw_bf[0:32, 1, g4:L:4])
                    dg1 = wk.tile([128, 5, 32], BF16, tag="dg1", name="dg1")
                    sid_b = bass.AP(tensor=sid4.tensor, offset=sid4[:, :].offset,
                                    ap=[sid4[:, :].ap[0], [0, 5], sid4[:, :].ap[1]])
                    nc.vector.tensor_mul(dg1[:, :, :], bcast_in(qw_pk[:, :], 32),
                                         sid_b)
                    for half in range(2):
                        pq = psb.tile([128, 2, BS], F32, tag="bq", name="pq")
                        mms = []
                        for ci in range(2):
                            c = half * 2 + ci
                            cs = slice(c * 128, (c + 1) * 128)
                            for l in range(L):
                                mms.append((pq[:, ci, 0:128], qv_b0[:, l, cs],
                                            dg[:, l, :]))
                            for g in range(5):
                                mms.append((pq[:, ci, 128:160], qv_p1[:, g, cs],
                                            dg1[:, g, :]))
                        for i, (o, lh, rh) in enumerate(mms):
                            nc.tensor.matmul(o, lh, rh, start=(i == 0),
                                             stop=(i == len(mms) - 1))
                        for ci in range(2):
                            c = half * 2 + ci
                            nc.scalar.activation(qcT[:, c, :], pq[:, ci, :],
                                                 AF.Identity,
                                                 bias=s_qvb[:, c:c + 1])

                    # --- step-19 argmax tail (emitted before gru_finish of
                    # step 20 so its DVE scans overlap step-20 attention) ---
                    if t == MAX_LEN - 1:
                        tail_emit()
                        gru_finish(rza, pend, bns, nxt, s_dbias,
                                   mid_cb=tail_trs, mid_at=18)
                    else:
                        gru_finish(rza, pend, bns, nxt, s_dbias)

                    # --- output projection ---
                    for bt in range(2):
                        pb, off = PBS[bt], BOFF[bt]
                        sl = slice(off, off + pb)
                        pso = pss.tile([128, 512], F32, tag="s", name="pso")
                        pairs = [(nxt[:, k, sl], s_out[:, k, :]) for k in range(4)]
                        pairs += [(qcT[:, k, sl], s_out[:, 4 + k, :])
                                  for k in range(4)]
                        pairs += [(iwT[:, k, sl], p2[:, k, :]) for k in range(2)]
                        pairs.append((ones_bf[:, :pb], s_outb[:]))
                        emit_group(pso[:pb, :D], pairs)
                        osb = st.tile([128, D], F32, tag="osb", name="osb")
                        nc.scalar.copy(osb[:pb], pso[:pb, :D])
                        nc.sync.dma_start(out_o[t, off:off + pb, :], osb[:pb])
                        if t == MAX_LEN - 2:
                            nc.vector.tensor_copy(o19_sb[bt][:pb], osb[:pb])

                    # --- prep for the argmax tail: o19 transposed ---
                    if t == MAX_LEN - 2:
                        for bt in range(2):
                            pb, off = PBS[bt], BOFF[bt]
                            ob = st.tile([128, D], BF16, tag="ob", name="ob")
                            nc.scalar.copy(ob[:pb], o19_sb[bt][:pb])
                            for kt in range(3):
                                w = XROWS[kt]
                                tr(o19T[:w, kt, off:off + pb],
                                   ob[:pb, kt * 128:kt * 128 + w], pb, w)

    nc.compile()
    return nc


_NC_CACHE = None


def _get_nc():
    global _NC_CACHE
    if _NC_CACHE is None:
        _NC_CACHE = build_nc()
    return _NC_CACHE


def _pad_tiles(a, ntiles):
    rows, cols = a.shape
    out = np.zeros((128 * ntiles, cols), a.dtype)
    out[:rows] = a
    return np.ascontiguousarray(out.reshape(ntiles, 128, cols).transpose(1, 0, 2))


def _bias_cols(vec):
    """[1536] -> [128, 12] (col = gate*4 + chunk, partition = feature%128)"""
    return np.ascontiguousarray(vec.reshape(12, 128).T)


def _prep_shared(inputs):
    bf = np.float16
    f32 = np.float32
    eW = np.asarray(inputs["embed_W"], f32)
    d = {}
    wih = np.asarray(inputs["dec_W_ih"], f32)
    whh = np.asarray(inputs["dec_W_hh"], f32)
    bih = np.asarray(inputs["dec_b_ih"], f32)
    bhh = np.asarray(inputs["dec_b_hh"], f32)
    ivW = np.asarray(inputs["iv_W"], f32)
    ivb = np.asarray(inputs["iv_b"], f32)
    outW = np.asarray(inputs["out_W"], f32)
    outb_v = np.asarray(inputs["out_b"], f32)

    d["w_gx"] = _pad_tiles(wih[:, 0:D].T.astype(bf), 3)
    d["w_gq"] = _pad_tiles(wih[:, D:D + H].T.astype(bf), 4)
    ghT = whh.T.copy()
    ghT[:, 2 * H:] *= 0.5
    d["w_gh"] = _pad_tiles(ghT.astype(bf), 4)
    Wic = wih[:, D + H:]                      # [3H, H]
    A = Wic @ ivW                             # [3H, 256]
    d["w_at"] = _pad_tiles(np.ascontiguousarray(A.T).astype(bf), 2)
    icb = Wic @ ivb                           # folded img bias [3H]
    db = np.zeros((128, 16), f32)
    rzb = 0.5 * (bih + bhh + icb)
    db[:, 0:4] = rzb[0:H].reshape(4, 128).T
    db[:, 4:8] = rzb[H:2 * H].reshape(4, 128).T
    db[:, 8:12] = (bih + icb)[2 * H:].reshape(4, 128).T
    db[:, 12:16] = (0.5 * bhh[2 * H:]).reshape(4, 128).T
    d["dbias"] = db

    ewih = np.asarray(inputs["enc_W_ih"], f32)
    ewhh = np.asarray(inputs["enc_W_hh"], f32)
    ebih = np.asarray(inputs["enc_b_ih"], f32)
    ebhh = np.asarray(inputs["enc_b_hh"], f32)
    d["w_egx"] = _pad_tiles(ewih[:, 0:D].T.astype(bf), 3)
    eghT = ewhh.T.copy()
    eghT[:, 2 * H:] *= 0.5
    d["w_egh"] = _pad_tiles(eghT.astype(bf), 4)
    eb = np.zeros((128, 16), f32)
    erzb = 0.5 * (ebih + ebhh)
    eb[:, 0:4] = erzb[0:H].reshape(4, 128).T
    eb[:, 4:8] = erzb[H:2 * H].reshape(4, 128).T
    eb[:, 8:12] = ebih[2 * H:].reshape(4, 128).T
    eb[:, 12:16] = (0.5 * ebhh[2 * H:]).reshape(4, 128).T
    d["ebias"] = eb

    d["w_out"] = _pad_tiles(outW[:, 0:2 * H].T.astype(bf), 8)
    Woic = outW[:, 2 * H:]                    # [300, H]
    A2 = Woic @ ivW                           # [300, 256]
    d["w_a2t"] = _pad_tiles(np.ascontiguousarray(A2.T).astype(bf), 2)
    d["outb"] = np.ascontiguousarray(
        (outb_v + Woic @ ivb).astype(bf)[None, :])

    d["w_qk"] = _pad_tiles(np.asarray(inputs["qk_W"], f32).T.astype(bf), 4)
    d["qkb"] = np.ascontiguousarray(
        np.asarray(inputs["qk_b"], f32).astype(bf)[None, :])
    d["w_qv"] = _pad_tiles(np.asarray(inputs["qv_W"], f32).T.astype(bf), 4)
    d["qvb_c"] = np.ascontiguousarray(
        np.asarray(inputs["qv_b"], f32).reshape(4, 128).T)
    d["w_ak"] = _pad_tiles(np.asarray(inputs["ak_W"], f32).T.astype(bf), 4)
    d["akb"] = np.ascontiguousarray(
        np.asarray(inputs["ak_b"], f32).astype(bf)[None, :])
    d["w_ik"] = _pad_tiles(np.asarray(inputs["ik_W"], f32).T.astype(bf), 2)
    ikb = np.zeros((128, 1), f32)
    ikb[:K, 0] = np.asarray(inputs["ik_b"], f32)
    d["ikb_c"] = ikb
    d["emb_bf"] = eW.astype(bf)
    wd_b = np.asarray(inputs["wd_b"], f32)
    d["emb_aug"] = np.ascontiguousarray(np.concatenate([eW, wd_b[:, None]], 1))
    aug = np.zeros((128 * 3, VP), f32)
    aug[:D, :V] = eW.T
    aug[320, :V] = wd_b
    d["embt_bf"] = _pad_tiles(aug.astype(bf), 3)
    return d


def _idx_cols(seq_rows):
    out = np.zeros((128, 2 * L), np.uint32)
    for t in range(L):
        out[:, 2 * t] = seq_rows[0:128, t]
        out[:32, 2 * t + 1] = seq_rows[128:160, t]
    return out


def _build_maps(inputs, shared):
    f32 = np.float32
    bf = np.float16
    ques = np.asarray(inputs["ques_seqs"]).astype(np.uint32)
    ans = np.asarray(inputs["ans_seqs"]).astype(np.uint32)
    qlens = np.asarray(inputs["ques_lens"]).astype(np.int64)
    img = np.asarray(inputs["img_seqs"], f32)
    maps = []
    for s in range(NCORES):
        m = dict(shared)
        r0 = s * BS
        m["q_idx"] = _idx_cols(ques[r0:r0 + BS, :L])
        m["a_idx"] = _idx_cols(ans[r0:r0 + BS, :L])
        qm = np.full((128, 2, L), -60000.0, bf)
        lens = qlens[r0:r0 + BS]
        for bt, (pb, off) in enumerate(zip(PBS, BOFF)):
            for b in range(pb):
                qm[b, bt, :lens[off + b]] = 0.0
        m["qe_mask"] = qm
        im = np.full((128, 2, IL), -60000.0, bf)
        for bt, (pb, off) in enumerate(zip(PBS, BOFF)):
            for b in range(pb):
                gimg = (off + b) // ROUNDS
                im[b, bt, gimg * 16:(gimg + 1) * 16] = 0.0
        m["ie_mask"] = im
        imgs = img[s * 16:(s + 1) * 16].reshape(IL, 256)
        it = np.zeros((128 * 2, IL), f32)
        it[:256] = imgs.T
        m["img_t"] = np.ascontiguousarray(
            it.reshape(2, 128, IL).transpose(1, 0, 2)).astype(bf)
        maps.append(m)
    return maps


def kernel(**inputs):
    nc = _get_nc()
    shared = _prep_shared(inputs)
    in_maps = _build_maps(inputs, shared)
    from concourse.bass_utils import run_bass_kernel_spmd
    res = run_bass_kernel_spmd(nc, in_maps, core_ids=list(range(NCORES)))
    outs = []
    for s in range(NCORES):
        o = np.asarray(res.results[s]["out_o"])
        outs.append(np.ascontiguousarray(o.transpose(1, 0, 2)))
    return np.concatenate(outs, 0).astype(np.float32)


# revision 23
# speedup vs baseline: 1.0952x; 1.0177x over previous
"""Trainium2 Bass kernel for nn_BaselineAttnDecoder.

Data-parallel over 8 NeuronCores: each core handles 160 decode rows
(= 16 images x 10 rounds). All weights replicated.

Feature-major GRU: weights are the stationary matmul operand, the 160
batch rows stream as moving columns, so gates land directly in the
transposed layout the next step needs (no per-step h transposes).
Image attention is folded: P = (W_ih_ic @ iv_W) @ img^T and
P2 = (W_out_ic @ iv_W) @ img^T are computed once on device, so the
image context vector is never materialized; its bias contribution
(softmax weights sum to 1) is folded into gate/output biases on host.
Sigmoid is computed as 0.5 + 0.5*tanh(x/2) (with W_hh_n pre-halved) so
the whole kernel uses one activation table (exp_and_others).
The transposed embedding table for the step-19 logits is persisted in
SBUF (loaded during the encoder), with bf16 top-8 + exact f32 rescore
for the argmax re-embedding.
"""
import numpy as np
import ml_dtypes

import concourse.bass as bass
import concourse.bacc as bacc
import concourse.mybir as mybir
import concourse.tile as tile
from concourse.masks import make_identity

F32 = mybir.dt.float32
BF16 = mybir.dt.float16  # 16-bit compute dtype (f16: 10-bit mantissa)
U32 = mybir.dt.uint32
AF = mybir.ActivationFunctionType
ALU = mybir.AluOpType
AX = mybir.AxisListType

D, H, V, K = 300, 512, 8835, 50
L, MAX_LEN, ROUNDS = 20, 21, 10
BS = 160
NCORES = 8
PBS = [128, 32]
BOFF = [0, 128]
IL = 256
VP = 8960
NEG = -1.0e30
G3 = 3 * H
XROWS = [128, 128, D - 256]  # k-tile partition sizes for embeddings


def bcast_mid(ap, reps):
    return bass.AP(tensor=ap.tensor, offset=ap.offset,
                   ap=[ap.ap[0], [0, reps], ap.ap[1]])


def bcast_in(ap, reps):
    return bass.AP(tensor=ap.tensor, offset=ap.offset,
                   ap=[ap.ap[0], ap.ap[1], [0, reps]])


def build_nc():
    nc = bacc.Bacc()

    def din(name, shape, dt):
        return nc.dram_tensor(name, shape, dt, kind="ExternalInput")

    # decoder GRU weights (feature-major stationary tiles)
    w_gx = din("w_gx", [128, 3, G3], BF16)      # x side (D rows)
    w_gq = din("w_gq", [128, 4, G3], BF16)      # q-context side (H rows)
    w_gh = din("w_gh", [128, 4, G3], BF16)      # hidden side (n cols halved)
    w_at = din("w_at", [128, 2, G3], BF16)      # (Wic@ivW).T img-feat rows
    dbias = din("dbias", [128, 16], F32)
    # encoder GRU weights
    w_egx = din("w_egx", [128, 3, G3], BF16)
    w_egh = din("w_egh", [128, 4, G3], BF16)
    ebias = din("ebias", [128, 16], F32)
    # output projection
    w_out = din("w_out", [128, 8, D], BF16)     # h + qc rows
    w_a2t = din("w_a2t", [128, 2, D], BF16)     # (Wout_ic@ivW).T
    outb = din("outb", [1, D], BF16)            # + Wout_ic@iv_b
    # attention projections
    w_qk = din("w_qk", [128, 4, K], BF16)
    qkb = din("qkb", [1, K], BF16)
    w_qv = din("w_qv", [128, 4, H], BF16)
    qvb_c = din("qvb_c", [128, 4], F32)
    w_ak = din("w_ak", [128, 4, K], BF16)
    akb = din("akb", [1, K], BF16)
    w_ik = din("w_ik", [128, 2, K], BF16)
    ikb_c = din("ikb_c", [128, 1], F32)
    img_t = din("img_t", [128, 2, IL], BF16)
    # embeddings
    emb_bf = din("emb_bf", [V, D], BF16)
    emb_aug = din("emb_aug", [V, D + 1], F32)
    embt_bf = din("embt_bf", [128, 3, VP], BF16)
    # per-core indices and masks
    q_idx = din("q_idx", [128, 2 * L], U32)
    a_idx = din("a_idx", [128, 2 * L], U32)
    qe_mask = din("qe_mask", [128, 2, L], BF16)
    ie_mask = din("ie_mask", [128, 2, IL], BF16)

    out_o = nc.dram_tensor("out_o", [MAX_LEN, BS, D], F32, kind="ExternalOutput")

    with tile.TileContext(nc) as tc:
        with (
            tc.tile_pool(name="cw", bufs=1) as cw,
            tc.tile_pool(name="pers", bufs=1) as pers,
            tc.tile_pool(name="wk", bufs=2) as wk,
            tc.tile_pool(name="st", bufs=2) as st,
            tc.tile_pool(name="psz", bufs=4, space="PSUM") as psz,
            tc.tile_pool(name="psb", bufs=2, space="PSUM") as psb,
            tc.tile_pool(name="pss", bufs=2, space="PSUM") as pss,
        ):
            def load(pool, t, dt):
                s = pool.tile(list(t.shape), dt, name=t.name + "_sb")
                nc.sync.dma_start(s[:], t[:])
                return s

            s_ak = load(cw, w_ak, BF16)
            s_ik = load(cw, w_ik, BF16)
            s_outb = load(cw, outb, BF16)
            s_akb = load(cw, akb, BF16)
            s_ikb = load(cw, ikb_c, F32)
            s_qvb = load(cw, qvb_c, F32)
            s_dbias = load(cw, dbias, F32)
            s_ebias = load(cw, ebias, F32)
            s_aidx = load(cw, a_idx, U32)
            s_qem = load(cw, qe_mask, BF16)
            s_iem = load(cw, ie_mask, BF16)
            # big persistent embedding-transpose table (used at step 19);
            # its DMA is issued inside the encoder so it doesn't delay startup
            s_embt = cw.tile([128, 3, VP], BF16, name="s_embt")

            ident_bf = cw.tile([128, 128], BF16)
            make_identity(nc, ident_bf[:])
            ones_bf = cw.tile([1, 128], BF16)
            nc.vector.memset(ones_bf[:], 1.0)
            sid4 = cw.tile([128, 32], BF16)
            for g4 in range(4):
                nc.vector.tensor_copy(sid4[32 * g4:32 * (g4 + 1), :],
                                      ident_bf[0:32, 0:32])
            iota8 = cw.tile([128, 8], F32)
            nc.gpsimd.iota(iota8[:], pattern=[[1, 8]], base=0, channel_multiplier=0,
                           allow_small_or_imprecise_dtypes=True)

            # persistent state (double-buffered hT)
            hT0 = pers.tile([128, 4, BS], BF16)
            hT1 = pers.tile([128, 4, BS], BF16)
            hTs = [hT0, hT1]
            h_f = pers.tile([128, 4, BS], F32)
            qk_b = pers.tile([128, 2, L, K], BF16)
            qv_b0 = pers.tile([128, L, H], BF16)
            qv_p1 = pers.tile([128, 5, H], BF16)
            ikt = pers.tile([128, IL], BF16)
            ptab = pers.tile([128, 2, G3], BF16)
            p2 = pers.tile([128, 2, D], BF16)
            a_bf = pers.tile([128, 2, K], BF16)
            qcT = pers.tile([128, 4, BS], BF16)
            iwT = pers.tile([128, 2, BS], BF16)
            dec20 = pers.tile([128, 3, BS], BF16)

            nc.vector.memset(hT0[:], 0.0)
            nc.vector.memset(h_f[:], 0.0)
            nc.vector.memset(qk_b[:], 0.0)
            nc.vector.memset(a_bf[:], 0.0)

            def tr(dst_sb_ap, src_sb_ap, pb, w, eng=None):
                pt = pss.tile([128, 128], BF16, tag="s", name="pt")
                nc.tensor.transpose(pt[:w, :pb], src_sb_ap, ident_bf[:pb, :pb])
                (eng or nc.vector).tensor_copy(dst_sb_ap, pt[:w, :pb])

            def fetch_x(idx_sb, t):
                xt = wk.tile([128, 3, BS], BF16, tag="xt", bufs=3, name="xt")
                for c, (pb, off) in enumerate(zip(PBS, BOFF)):
                    g = wk.tile([128, D], BF16, tag="gath", bufs=4, name="g")
                    nc.gpsimd.indirect_dma_start(
                        out=g[:pb], out_offset=None, in_=emb_bf[:],
                        in_offset=bass.IndirectOffsetOnAxis(
                            ap=idx_sb[:pb, 2 * t + c:2 * t + c + 1], axis=0))
                    for kt in range(3):
                        w = XROWS[kt]
                        tr(xt[:w, kt, off:off + pb], g[:pb, kt * 128:kt * 128 + w],
                           pb, w)
                return xt

            def emit_group(ps_ap, pairs):
                n = len(pairs)
                for i, (lh, rh) in enumerate(pairs):
                    nc.tensor.matmul(ps_ap, lh, rh, start=(i == 0), stop=(i == n - 1))

            # ---------------- one-time precompute ----------------
            with tc.tile_pool(name="pre", bufs=1) as pre:
                s_imgt = load(pre, img_t, BF16)
                s_at = load(pre, w_at, BF16)
                s_a2t = load(pre, w_a2t, BF16)
                # ikt[K, IL] = image keys (transposed)
                psik = pss.tile([128, IL], F32, tag="s", name="psik")
                emit_group(psik[:K, :], [(s_ik[:, kt, :], s_imgt[:, kt, :])
                                         for kt in range(2)])
                nc.vector.tensor_scalar_add(ikt[:K, :], psik[:K, :], s_ikb[:K, :])

                # ptab[j, 1536] = (Wic@ivW @ img^T)^T tiles; p2[j, 300] likewise
                for jt in range(2):
                    for ch in range(3):
                        psp = pss.tile([128, 512], F32, tag="s", name="psp")
                        emit_group(psp[:, :],
                                   [(s_imgt[:, kt, jt * 128:(jt + 1) * 128],
                                     s_at[:, kt, ch * 512:(ch + 1) * 512])
                                    for kt in range(2)])
                        nc.scalar.copy(ptab[:, jt, ch * 512:(ch + 1) * 512],
                                       psp[:, :])
                    psp2 = pss.tile([128, 512], F32, tag="s", name="psp2")
                    emit_group(psp2[:, :D],
                               [(s_imgt[:, kt, jt * 128:(jt + 1) * 128],
                                 s_a2t[:, kt, :]) for kt in range(2)])
                    nc.scalar.copy(p2[:, jt, :], psp2[:, :D])

            # ---------------- feature-major GRU core ----------------
            def gru_fm(cur, nxt, gi_srcs, gh_w, bias_sb):
                """gi_srcs: list of (weight_sb, src_fn(kt)->AP, nkt, rows)
                covering x (+ qc/img for decoder); gh_w hidden weights with
                n-columns pre-halved; bias cols: [r0..3, z0..3, n0..3, bn0..3].
                Updates h_f in place and writes nxt (hT double buffer)."""
                rza = [psz.tile([128, 3, BS], F32, tag="rza", name="rza")
                       for _ in range(4)]
                # early members per chunk: hidden side ordered kt-major so the
                # PE can start as soon as each hT chunk of the previous step
                # lands, then x/img sides; late (attention-dependent) members
                # close the groups in gru_finish.
                pend = [[] for _ in range(4)]
                early = [[] for _ in range(4)]
                for kt in range(4):
                    for c in range(4):
                        for g in range(2):  # r, z hidden side
                            cs = slice(g * H + c * 128, g * H + (c + 1) * 128)
                            early[c].append((rza[c][:, g, :], gh_w[:, kt, cs],
                                             cur[:, kt, :]))
                for c in range(4):
                    for (wsb, srcf, nkt, rows, late) in gi_srcs:
                        for g in range(3):
                            cs = slice(g * H + c * 128, g * H + (c + 1) * 128)
                            for kt in range(nkt):
                                r = rows[kt]
                                mm = (rza[c][:, g, :], wsb[:r, kt, cs],
                                      srcf(kt, r))
                                (pend[c] if late else early[c]).append(mm)
                started = [False] * 4
                emit_seq = []
                for kt in range(4):
                    for c in range(4):
                        emit_seq.append((c, early[c][2 * kt]))
                        emit_seq.append((c, early[c][2 * kt + 1]))
                for c in range(4):
                    for mm in early[c][8:]:
                        emit_seq.append((c, mm))
                remaining = [len(early[c]) for c in range(4)]
                for c, (o, lh, rh) in emit_seq:
                    remaining[c] -= 1
                    nc.tensor.matmul(o, lh, rh, start=(not started[c]),
                                     stop=(not pend[c] and remaining[c] == 0))
                    started[c] = True
                # bn: hidden-side n gate (0.5-scaled weights); extract to SBUF
                # immediately (with bias) so the psum banks free up for qc
                bnbs = []
                for half in range(2):
                    bn = psb.tile([128, 2, BS], F32, tag="bq", name="bn")
                    pairs = []
                    for ci in range(2):
                        c = half * 2 + ci
                        cs = slice(2 * H + c * 128, 2 * H + (c + 1) * 128)
                        pairs += [(gh_w[:, kt, cs], cur[:, kt, :])
                                  for kt in range(4)]
                    for i, (lh, rh) in enumerate(pairs):
                        nc.tensor.matmul(bn[:, i // 4, :], lh, rh,
                                         start=(i == 0), stop=(i == len(pairs) - 1))
                    for ci in range(2):
                        c = half * 2 + ci
                        bnb = st.tile([128, BS], F32, tag="bnb", bufs=4,
                                      name="bnb")
                        nc.gpsimd.tensor_scalar_add(bnb[:], bn[:, ci, :],
                                                    bias_sb[:, 12 + c:13 + c])
                        bnbs.append(bnb)
                return rza, pend, bnbs

            def gru_finish(rza, pend, bnbs, nxt, bias_sb, mid_cb=None,
                           mid_at=None):
                # late members close each chunk's group, chunk-major so chunk
                # 0's activations overlap chunk 1's matmuls. mid_cb is emitted
                # between member index mid_at and the rest (used at t=20 to
                # place the dec20 transposes before the x members).
                for c in range(4):
                    for i, (ps_ap, lh, rh) in enumerate(pend[c][:mid_at]):
                        nc.tensor.matmul(ps_ap, lh, rh, start=False,
                                         stop=(i == len(pend[c]) - 1))
                if mid_cb is not None:
                    mid_cb()
                if mid_at is not None:
                    for c in range(4):
                        for j, (ps_ap, lh, rh) in enumerate(pend[c][mid_at:]):
                            i = mid_at + j
                            nc.tensor.matmul(ps_ap, lh, rh, start=False,
                                             stop=(i == len(pend[c]) - 1))
                for c in range(4):
                    ps = rza[c]
                    bnb = bnbs[c]
                    th_r = st.tile([128, BS], F32, tag="thr", name="th_r")
                    nc.scalar.activation(th_r[:], ps[:, 0, :], AF.Tanh,
                                         bias=bias_sb[:, c:c + 1], scale=0.5)
                    th_z = st.tile([128, BS], F32, tag="thz", name="th_z")
                    nc.scalar.activation(th_z[:], ps[:, 1, :], AF.Tanh,
                                         bias=bias_sb[:, 4 + c:5 + c], scale=0.5)
                    t1 = st.tile([128, BS], F32, tag="t1", name="t1")
                    nc.vector.scalar_tensor_tensor(t1[:], th_r[:], 1.0, bnb[:],
                                                   op0=ALU.add, op1=ALU.mult)
                    nc.vector.tensor_add(t1[:], t1[:], ps[:, 2, :])
                    n = st.tile([128, BS], F32, tag="n", name="n")
                    nc.scalar.activation(n[:], t1[:], AF.Tanh,
                                         bias=bias_sb[:, 8 + c:9 + c])
                    s1 = st.tile([128, BS], F32, tag="s1", name="s1")
                    nc.gpsimd.tensor_sub(s1[:], h_f[:, c, :], n[:])
                    u = st.tile([128, BS], F32, tag="u", name="u")
                    nc.vector.scalar_tensor_tensor(u[:], th_z[:], 1.0, s1[:],
                                                   op0=ALU.add, op1=ALU.mult)
                    nc.vector.scalar_tensor_tensor(h_f[:, c, :], u[:], 0.5, n[:],
                                                   op0=ALU.mult, op1=ALU.add)
                    nc.gpsimd.tensor_copy(nxt[:, c, :], h_f[:, c, :])

            # ---------------- encoder ----------------
            with tc.tile_pool(name="qp", bufs=1) as qp:
                s_egx = load(qp, w_egx, BF16)
                s_egh = load(qp, w_egh, BF16)
                s_qk = load(qp, w_qk, BF16)
                s_qv = load(qp, w_qv, BF16)
                s_qkb = load(qp, qkb, BF16)
                s_qidx = load(qp, q_idx, U32)
                for kt in range(3):
                    nc.sync.dma_start(s_embt[:, kt, :], embt_bf[:, kt, :])
                def qkv_proj(t, ht):
                    # qk / qv projections from h at encoder step t
                    for bt in range(2):
                        pb, off = PBS[bt], BOFF[bt]
                        sl = slice(off, off + pb)
                        psk = pss.tile([128, 512], F32, tag="s", name="psk")
                        pairs = [(ht[:, kt, sl], s_qk[:, kt, :]) for kt in range(4)]
                        pairs.append((ones_bf[:, :pb], s_qkb[:]))
                        emit_group(psk[:pb, :K], pairs)
                        nc.scalar.copy(qk_b[:pb, bt, t, :], psk[:pb, :K])
                        psv = pss.tile([128, 512], F32, tag="s", name="psv")
                        emit_group(psv[:pb, :],
                                   [(ht[:, kt, sl], s_qv[:, kt, :])
                                    for kt in range(4)])
                        if bt == 0:
                            nc.scalar.copy(qv_b0[:pb, t, :], psv[:pb, :])
                        else:
                            g4 = t % 4
                            nc.scalar.copy(
                                qv_p1[32 * g4:32 * (g4 + 1), t // 4, :],
                                psv[:pb, :])

                xt_n = fetch_x(s_qidx, 0)
                for t in range(L):
                    cur, nxt = hTs[t % 2], hTs[(t + 1) % 2]
                    xt = xt_n
                    srcs = [(s_egx, lambda kt, r, xt=xt: xt[:r, kt, :], 3,
                             XROWS, False)]
                    rza, pend, bns = gru_fm(cur, nxt, srcs, s_egh, s_ebias)
                    # previous step's projections fill the PE while this
                    # step's activation chain drains
                    if t > 0:
                        qkv_proj(t - 1, cur)
                    if t + 1 < L:
                        xt_n = fetch_x(s_qidx, t + 1)
                    gru_finish(rza, pend, bns, nxt, s_ebias)
                qkv_proj(L - 1, hTs[L % 2])

            nc.vector.memset(hT0[:], 0.0)
            nc.vector.memset(h_f[:], 0.0)

            # ---------------- decoder ----------------
            with tc.tile_pool(name="lg", bufs=1) as lg:
                s_gx = load(lg, w_gx, BF16)
                s_gq = load(lg, w_gq, BF16)
                s_gh = load(lg, w_gh, BF16)
                s_out = load(lg, w_out, BF16)
                o19T = lg.tile([128, 3, BS], BF16)
                nc.vector.memset(o19T[32:64, 2, :], 0.0)
                nc.vector.memset(o19T[64:65, 2, :], 1.0)
                o19_0 = lg.tile([128, D], F32)
                o19_1 = lg.tile([128, D], F32)
                o19_sb = [o19_0, o19_1]
                RW = 18 * 256               # pairwise-reduced scan width
                NPAIR = (V + 1) // 2        # 4418 valid pairs
                red0 = lg.tile([128, RW], BF16)
                red1 = lg.tile([128, RW], BF16)
                reds = [red0, red1]

                def tail_logits(bt, second):
                    """logit blocks on PE; pairwise max straight out of PSUM
                    into the reduced scan buffer (no full logit storage)."""
                    pb, off = PBS[bt], BOFF[bt]
                    red = reds[bt]
                    nc.vector.memset(red[:, 2 * NPAIR // 2:], -60000.0)
                    for nci in range(18):
                        ncw = 512 if nci < 17 else V - 17 * 512
                        psl = pss.tile([128, 512], F32, tag="s", name="psl")
                        pairs = []
                        for kt in range(3):
                            nr = 128 if kt < 2 else 65
                            pairs.append((o19T[:nr, kt, off:off + pb],
                                          s_embt[:nr, kt,
                                                 nci * 512:nci * 512 + ncw]))
                        emit_group(psl[:pb, :ncw], pairs)
                        ro = nci * 256
                        npr = ncw // 2
                        eng = nc.gpsimd if nci % 3 == 1 else nc.vector
                        eng.tensor_tensor(red[:pb, ro:ro + npr],
                                          psl[:pb, 0:2 * npr:2],
                                          psl[:pb, 1:2 * npr:2], op=ALU.max)
                        if ncw % 2:
                            nc.vector.tensor_copy(
                                red[:pb, ro + npr:ro + npr + 1],
                                psl[:pb, ncw - 1:ncw])

                def tail_scan(bt, first):
                    pb = PBS[bt]
                    red = reds[bt]
                    mx8 = st.tile([128, 8], BF16, tag="mx8", name="mx8")
                    nc.vector.max(mx8[:pb], red[:pb])
                    ix8 = st.tile([128, 8], U32, tag="ix8", name="ix8")
                    nc.vector.max_index(ix8[:pb], mx8[:pb], red[:pb])
                    return ix8

                def tail_rescore(bt, ix8):
                    """top-4 pairs -> 8 vocab candidates (measured margin:
                    the f32 argmax is always in the top-2 f16 logits), exact
                    f32 rescore on Pool."""
                    pb, off = PBS[bt], BOFF[bt]
                    cand = st.tile([128, 8], F32, tag="cand", name="cand")
                    nc.gpsimd.tensor_copy(cand[:pb, 0:4], ix8[:pb, 0:4])
                    nc.gpsimd.tensor_scalar_mul(cand[:pb, 0:4], cand[:pb, 0:4],
                                                2.0)
                    nc.gpsimd.tensor_scalar_add(cand[:pb, 4:8],
                                                cand[:pb, 0:4], 1.0)
                    nc.gpsimd.tensor_scalar_min(cand[:pb], cand[:pb],
                                                float(V - 1))
                    cand_u = st.tile([128, 8], U32, tag="candu", name="cand_u")
                    nc.gpsimd.tensor_copy(cand_u[:pb], cand[:pb])
                    scores = st.tile([128, 8], F32, tag="sco", name="scores")
                    g8 = wk.tile([128, 8, D + 1], F32, tag="gath8", bufs=1,
                                 name="g8")
                    nc.gpsimd.indirect_dma_start(
                        out=g8[:pb], out_offset=None, in_=emb_aug[:],
                        in_offset=bass.IndirectOffsetOnAxis(
                            ap=cand_u[:pb, 0:8], axis=0))
                    for j in range(8):
                        pr = wk.tile([128, D], F32, tag="pr8", bufs=2,
                                     name="pr")
                        sj = st.tile([128, 1], F32, tag="sj", name="sj")
                        nc.gpsimd.scalar_tensor_tensor(
                            pr[:pb], o19_sb[bt][:pb], 1.0, g8[:pb, j, :D],
                            op0=ALU.mult, op1=ALU.mult, accum_out=sj[:pb])
                        nc.gpsimd.tensor_add(scores[:pb, j:j + 1],
                                             sj[:pb], g8[:pb, j, D:D + 1])
                    # argmax over the 8 rescored candidates (min idx on tie)
                    m4 = st.tile([128, 4], F32, tag="m4", name="m4")
                    nc.gpsimd.tensor_max(m4[:pb], scores[:pb, 0:4],
                                         scores[:pb, 4:8])
                    nc.gpsimd.tensor_max(m4[:pb, 0:2], m4[:pb, 0:2],
                                         m4[:pb, 2:4])
                    mxs = st.tile([128, 1], F32, tag="mxs", name="mxs")
                    nc.gpsimd.tensor_max(mxs[:pb], m4[:pb, 0:1], m4[:pb, 1:2])
                    oh = st.tile([128, 8], F32, tag="oh", name="oh")
                    nc.gpsimd.tensor_scalar(out=oh[:pb], in0=scores[:pb],
                                            scalar1=mxs[:pb], scalar2=None,
                                            op0=ALU.is_equal)
                    sel = st.tile([128, 8], F32, tag="sel", name="sel")
                    nc.gpsimd.tensor_scalar_sub(sel[:pb], cand[:pb], 65536.0)
                    nc.gpsimd.tensor_mul(sel[:pb], oh[:pb], sel[:pb])
                    nc.gpsimd.tensor_scalar_add(sel[:pb], sel[:pb], 65536.0)
                    nc.gpsimd.tensor_tensor(sel[:pb, 0:4], sel[:pb, 0:4],
                                            sel[:pb, 4:8], op=ALU.min)
                    nc.gpsimd.tensor_tensor(sel[:pb, 0:2], sel[:pb, 0:2],
                                            sel[:pb, 2:4], op=ALU.min)
                    vsum = st.tile([128, 1], F32, tag="vsum", name="vsum")
                    nc.gpsimd.tensor_tensor(vsum[:pb], sel[:pb, 0:1],
                                            sel[:pb, 1:2], op=ALU.min)
                    vidx = st.tile([128, 1], U32, tag="vidx", name="vidx")
                    nc.gpsimd.tensor_copy(vidx[:pb], vsum[:pb])
                    gm = wk.tile([128, D], BF16, tag="gath", bufs=4, name="gm")
                    nc.gpsimd.indirect_dma_start(
                        out=gm[:pb], out_offset=None, in_=emb_bf[:],
                        in_offset=bass.IndirectOffsetOnAxis(
                            ap=vidx[:pb, 0:1], axis=0))
                    return gm

                tail_gms = {}

                def tail_emit():
                    tail_logits(1, second=False)
                    ix1 = tail_scan(1, first=True)
                    tail_logits(0, second=True)
                    ix0 = tail_scan(0, first=False)
                    tail_gms[1] = tail_rescore(1, ix1)
                    tail_gms[0] = tail_rescore(0, ix0)

                def tail_trs():
                    for bt in range(2):
                        pb, off = PBS[bt], BOFF[bt]
                        gm = tail_gms[bt]
                        for kt in range(3):
                            w = XROWS[kt]
                            tr(dec20[:w, kt, off:off + pb],
                               gm[:pb, kt * 128:kt * 128 + w], pb, w)

                xt_n = fetch_x(s_aidx, 0)
                for t in range(MAX_LEN):
                    cur, nxt = hTs[t % 2], hTs[(t + 1) % 2]
                    # --- attention query a = h@ak_W + ak_b ---
                    aT = st.tile([128, BS], BF16, tag="aT", name="aT")
                    for bt in range(2):
                        pb, off = PBS[bt], BOFF[bt]
                        sl = slice(off, off + pb)
                        psa = pss.tile([128, 512], F32, tag="s", name="psa")
                        pairs = [(cur[:, kt, sl], s_ak[:, kt, :]) for kt in range(4)]
                        pairs.append((ones_bf[:, :pb], s_akb[:]))
                        emit_group(psa[:pb, :K], pairs)
                        nc.scalar.copy(a_bf[:pb, bt, :], psa[:pb, :K])
                        tr(aT[:K, off:off + pb], a_bf[:pb, bt, :], pb, K)

                    # --- GRU early members (hidden + x sides) ---
                    xt = xt_n if t < L else dec20
                    late_x = (t == MAX_LEN - 1)
                    src_x = (s_gx, lambda kt, r, xt=xt: xt[:r, kt, :], 3,
                             XROWS, late_x)
                    srcs = [
                        (s_gq, lambda kt, r: qcT[:r, kt, :], 4, [128] * 4, True),
                        (ptab, lambda kt, r: iwT[:r, kt, :], 2, [128] * 2, True),
                    ]
                    srcs = srcs + [src_x] if late_x else [src_x] + srcs
                    rza, pend, bns = gru_fm(cur, nxt, srcs, s_gh, s_dbias)
                    if t + 1 < L:
                        xt_n = fetch_x(s_aidx, t + 1)

                    # --- question attention scores (DVE) ---
                    prod = wk.tile([128, 2, L, K], BF16, tag="prod", bufs=1,
                                   name="prod")
                    abc = bass.AP(tensor=a_bf.tensor, offset=a_bf[:, :, :].offset,
                                  ap=[a_bf[:, :, :].ap[0], a_bf[:, :, :].ap[1],
                                      [0, L], a_bf[:, :, :].ap[2]])
                    nc.vector.tensor_tensor(out=prod[:], in0=qk_b[:, :, :, :],
                                            in1=abc, op=ALU.mult)
                    qe = st.tile([128, 2, L], F32, tag="qe", name="qe")
                    nc.vector.tensor_reduce(qe[:], prod[:], axis=AX.X, op=ALU.add)
                    nc.vector.tensor_add(qe[:], qe[:], s_qem[:, :, :])
                    qw_bf = st.tile([128, 2, L], BF16, tag="qwb", name="qw_bf")
                    for bt in range(2):
                        pb = PBS[bt]
                        nm = st.tile([128, 1], F32, tag="nm", name="nm")
                        nc.vector.tensor_reduce(nm[:pb], qe[:pb, bt, :], axis=AX.X,
                                                op=ALU.max, negate=True)
                        ew = st.tile([128, L], F32, tag="ew", name="ew")
                        ssum = st.tile([128, 1], F32, tag="ssum", name="ssum")
                        nc.scalar.activation(ew[:pb], qe[:pb, bt, :], AF.Exp,
                                             bias=nm[:pb], scale=1.0,
                                             accum_out=ssum[:pb])
                        rs = st.tile([128, 1], F32, tag="rs", name="rs")
                        nc.vector.reciprocal(rs[:pb], ssum[:pb])
                        nc.vector.tensor_scalar_mul(qw_bf[:pb, bt, :], ew[:pb],
                                                    rs[:pb])

                    # --- image attention (needs aT) ---
                    for bt in range(2):
                        pb, off = PBS[bt], BOFF[bt]
                        psi = pss.tile([128, 512], F32, tag="s", name="psi")
                        nc.tensor.matmul(psi[:pb, :IL], aT[:K, off:off + pb],
                                         ikt[:K, :], start=True, stop=True)
                        iem = st.tile([128, IL], F32, tag="iem", name="iem")
                        nc.vector.tensor_add(iem[:pb], psi[:pb, :IL],
                                             s_iem[:pb, bt, :])
                        nmi = st.tile([128, 1], F32, tag="nmi", name="nmi")
                        nc.vector.tensor_reduce(nmi[:pb], iem[:pb], axis=AX.X,
                                                op=ALU.max, negate=True)
                        ewi = st.tile([128, IL], F32, tag="ewi", name="ewi")
                        ssi = st.tile([128, 1], F32, tag="ssi", name="ssi")
                        nc.scalar.activation(ewi[:pb], iem[:pb], AF.Exp,
                                             bias=nmi[:pb], scale=1.0,
                                             accum_out=ssi[:pb])
                        rsi = st.tile([128, 1], F32, tag="rsi", name="rsi")
                        nc.vector.reciprocal(rsi[:pb], ssi[:pb])
                        iwb = st.tile([128, IL], BF16, tag="iwb", name="iwb")
                        nc.vector.tensor_scalar_mul(iwb[:pb], ewi[:pb], rsi[:pb])
                        for c in range(2):
                            tr(iwT[:, c, off:off + pb],
                               iwb[:pb, c * 128:(c + 1) * 128], pb, 128,
                               eng=nc.gpsimd)

                    # --- question context, feature-major ---
                    dg = wk.tile([128, L, 128], BF16, tag="diag", bufs=1, name="dg")
                    hl = L // 2
                    ibh = ident_bf[:128, :128]
                    ident_h = bass.AP(tensor=ibh.tensor, offset=ibh.offset,
                                      ap=[ibh.ap[0], [0, hl], ibh.ap[1]])
                    nc.gpsimd.tensor_mul(dg[:, :hl, :],
                                         bcast_in(qw_bf[:, 0, :hl], 128), ident_h)
                    nc.vector.tensor_mul(dg[:, hl:, :],
                                         bcast_in(qw_bf[:, 0, hl:], 128), ident_h)
                    qw_pk = st.tile([128, 5], BF16, tag="qwpk", name="qw_pk")
                    for g4 in range(4):
                        nc.vector.tensor_copy(qw_pk[32 * g4:32 * (g4 + 1), :],
                                              qw_bf[0:32, 1, g4:L:4])
                    dg1 = wk.tile([128, 5, 32], BF16, tag="dg1", name="dg1")
                    sid_b = bass.AP(tensor=sid4.tensor, offset=sid4[:, :].offset,
                                    ap=[sid4[:, :].ap[0], [0, 5], sid4[:, :].ap[1]])
                    nc.vector.tensor_mul(dg1[:, :, :], bcast_in(qw_pk[:, :], 32),
                                         sid_b)
                    for half in range(2):
                        pq = psb.tile([128, 2, BS], F32, tag="bq", name="pq")
                        mms = []
                        for ci in range(2):
                            c = half * 2 + ci
                            cs = slice(c * 128, (c + 1) * 128)
                            for l in range(L):
                                mms.append((pq[:, ci, 0:128], qv_b0[:, l, cs],
                                            dg[:, l, :]))
                            for g in range(5):
                                mms.append((pq[:, ci, 128:160], qv_p1[:, g, cs],
                                            dg1[:, g, :]))
                        for i, (o, lh, rh) in enumerate(mms):
                            nc.tensor.matmul(o, lh, rh, start=(i == 0),
                                             stop=(i == len(mms) - 1))
                        for ci in range(2):
                            c = half * 2 + ci
                            nc.scalar.activation(qcT[:, c, :], pq[:, ci, :],
                                                 AF.Identity,
                                                 bias=s_qvb[:, c:c + 1])

                    # --- step-19 argmax tail (emitted before gru_finish of
                    # step 20 so its DVE scans overlap step-20 attention) ---
                    if t == MAX_LEN - 1:
                        tail_emit()
                        gru_finish(rza, pend, bns, nxt, s_dbias,
                                   mid_cb=tail_trs, mid_at=18)
                    else:
                        gru_finish(rza, pend, bns, nxt, s_dbias)

                    # --- output projection ---
                    for bt in range(2):
                        pb, off = PBS[bt], BOFF[bt]
                        sl = slice(off, off + pb)
                        pso = pss.tile([128, 512], F32, tag="s", name="pso")
                        pairs = [(nxt[:, k, sl], s_out[:, k, :]) for k in range(4)]
                        pairs += [(qcT[:, k, sl], s_out[:, 4 + k, :])
                                  for k in range(4)]
                        pairs += [(iwT[:, k, sl], p2[:, k, :]) for k in range(2)]
                        pairs.append((ones_bf[:, :pb], s_outb[:]))
                        emit_group(pso[:pb, :D], pairs)
                        osb = st.tile([128, D], F32, tag="osb", name="osb")
                        nc.scalar.copy(osb[:pb], pso[:pb, :D])
                        nc.sync.dma_start(out_o[t, off:off + pb, :], osb[:pb])
                        if t == MAX_LEN - 2:
                            nc.vector.tensor_copy(o19_sb[bt][:pb], osb[:pb])

                    # --- prep for the argmax tail: o19 transposed ---
                    if t == MAX_LEN - 2:
                        for bt in range(2):
                            pb, off = PBS[bt], BOFF[bt]
                            ob = st.tile([128, D], BF16, tag="ob", name="ob")
                            nc.scalar.copy(ob[:pb], o19_sb[bt][:pb])
                            for kt in range(3):
                                w = XROWS[kt]
                                tr(o19T[:w, kt, off:off + pb],
                                   ob[:pb, kt * 128:kt * 128 + w], pb, w)

    nc.compile()
    return nc


_NC_CACHE = None


def _get_nc():
    global _NC_CACHE
    if _NC_CACHE is None:
        _NC_CACHE = build_nc()
    return _NC_CACHE


def _pad_tiles(a, ntiles):
    rows, cols = a.shape
    out = np.zeros((128 * ntiles, cols), a.dtype)
    out[:rows] = a
    return np.ascontiguousarray(out.reshape(ntiles, 128, cols).transpose(1, 0, 2))


def _bias_cols(vec):
    """[1536] -> [128, 12] (col = gate*4 + chunk, partition = feature%128)"""
    return np.ascontiguousarray(vec.reshape(12, 128).T)


def _prep_shared(inputs):
    bf = np.float16
    f32 = np.float32
    eW = np.asarray(inputs["embed_W"], f32)
    d = {}
    wih = np.asarray(inputs["dec_W_ih"], f32)
    whh = np.asarray(inputs["dec_W_hh"], f32)
    bih = np.asarray(inputs["dec_b_ih"], f32)
    bhh = np.asarray(inputs["dec_b_hh"], f32)
    ivW = np.asarray(inputs["iv_W"], f32)
    ivb = np.asarray(inputs["iv_b"], f32)
    outW = np.asarray(inputs["out_W"], f32)
    outb_v = np.asarray(inputs["out_b"], f32)

    d["w_gx"] = _pad_tiles(wih[:, 0:D].T.astype(bf), 3)
    d["w_gq"] = _pad_tiles(wih[:, D:D + H].T.astype(bf), 4)
    ghT = whh.T.copy()
    ghT[:, 2 * H:] *= 0.5
    d["w_gh"] = _pad_tiles(ghT.astype(bf), 4)
    Wic = wih[:, D + H:]                      # [3H, H]
    A = Wic @ ivW                             # [3H, 256]
    d["w_at"] = _pad_tiles(np.ascontiguousarray(A.T).astype(bf), 2)
    icb = Wic @ ivb                           # folded img bias [3H]
    db = np.zeros((128, 16), f32)
    rzb = 0.5 * (bih + bhh + icb)
    db[:, 0:4] = rzb[0:H].reshape(4, 128).T
    db[:, 4:8] = rzb[H:2 * H].reshape(4, 128).T
    db[:, 8:12] = (bih + icb)[2 * H:].reshape(4, 128).T
    db[:, 12:16] = (0.5 * bhh[2 * H:]).reshape(4, 128).T
    d["dbias"] = db

    ewih = np.asarray(inputs["enc_W_ih"], f32)
    ewhh = np.asarray(inputs["enc_W_hh"], f32)
    ebih = np.asarray(inputs["enc_b_ih"], f32)
    ebhh = np.asarray(inputs["enc_b_hh"], f32)
    d["w_egx"] = _pad_tiles(ewih[:, 0:D].T.astype(bf), 3)
    eghT = ewhh.T.copy()
    eghT[:, 2 * H:] *= 0.5
    d["w_egh"] = _pad_tiles(eghT.astype(bf), 4)
    eb = np.zeros((128, 16), f32)
    erzb = 0.5 * (ebih + ebhh)
    eb[:, 0:4] = erzb[0:H].reshape(4, 128).T
    eb[:, 4:8] = erzb[H:2 * H].reshape(4, 128).T
    eb[:, 8:12] = ebih[2 * H:].reshape(4, 128).T
    eb[:, 12:16] = (0.5 * ebhh[2 * H:]).reshape(4, 128).T
    d["ebias"] = eb

    d["w_out"] = _pad_tiles(outW[:, 0:2 * H].T.astype(bf), 8)
    Woic = outW[:, 2 * H:]                    # [300, H]
    A2 = Woic @ ivW                           # [300, 256]
    d["w_a2t"] = _pad_tiles(np.ascontiguousarray(A2.T).astype(bf), 2)
    d["outb"] = np.ascontiguousarray(
        (outb_v + Woic @ ivb).astype(bf)[None, :])

    d["w_qk"] = _pad_tiles(np.asarray(inputs["qk_W"], f32).T.astype(bf), 4)
    d["qkb"] = np.ascontiguousarray(
        np.asarray(inputs["qk_b"], f32).astype(bf)[None, :])
    d["w_qv"] = _pad_tiles(np.asarray(inputs["qv_W"], f32).T.astype(bf), 4)
    d["qvb_c"] = np.ascontiguousarray(
        np.asarray(inputs["qv_b"], f32).reshape(4, 128).T)
    d["w_ak"] = _pad_tiles(np.asarray(inputs["ak_W"], f32).T.astype(bf), 4)
    d["akb"] = np.ascontiguousarray(
        np.asarray(inputs["ak_b"], f32).astype(bf)[None, :])
    d["w_ik"] = _pad_tiles(np.asarray(inputs["ik_W"], f32).T.astype(bf), 2)
    ikb = np.zeros((128, 1), f32)
    ikb[:K, 0] = np.asarray(inputs["ik_b"], f32)
    d["ikb_c"] = ikb
    d["emb_bf"] = eW.astype(bf)
    wd_b = np.asarray(inputs["wd_b"], f32)
    d["emb_aug"] = np.ascontiguousarray(np.concatenate([eW, wd_b[:, None]], 1))
    aug = np.zeros((128 * 3, VP), f32)
    aug[:D, :V] = eW.T
    aug[320, :V] = wd_b
    d["embt_bf"] = _pad_tiles(aug.astype(bf), 3)
    return d


def _idx_cols(seq_rows):
    out = np.zeros((128, 2 * L), np.uint32)
    for t in range(L):
        out[:, 2 * t] = seq_rows[0:128, t]
        out[:32, 2 * t + 1] = seq_rows[128:160, t]
    return out


def _build_maps(inputs, shared):
    f32 = np.float32
    bf = np.float16
    ques = np.asarray(inputs["ques_seqs"]).astype(np.uint32)
    ans = np.asarray(inputs["ans_seqs"]).astype(np.uint32)
    qlens = np.asarray(inputs["ques_lens"]).astype(np.int64)
    img = np.asarray(inputs["img_seqs"], f32)
    maps = []
    for s in range(NCORES):
        m = dict(shared)
        r0 = s * BS
        m["q_idx"] = _idx_cols(ques[r0:r0 + BS, :L])
        m["a_idx"] = _idx_cols(ans[r0:r0 + BS, :L])
        qm = np.full((128, 2, L), -60000.0, bf)
        lens = qlens[r0:r0 + BS]
        for bt, (pb, off) in enumerate(zip(PBS, BOFF)):
            for b in range(pb):
                qm[b, bt, :lens[off + b]] = 0.0
        m["qe_mask"] = qm
        im = np.full((128, 2, IL), -60000.0, bf)
        for bt, (pb, off) in enumerate(zip(PBS, BOFF)):
            for b in range(pb):
                gimg = (off + b) // ROUNDS
                im[b, bt, gimg * 16:(gimg + 1) * 16] = 0.0
        m["ie_mask"] = im
        imgs = img[s * 16:(s + 1) * 16].reshape(IL, 256)
        it = np.zeros((128 * 2, IL), f32)
        it[:256] = imgs.T
        m["img_t"] = np.ascontiguousarray(
            it.reshape(2, 128, IL).transpose(1, 0, 2)).astype(bf)
        maps.append(m)
    return maps


def kernel(**inputs):
    nc = _get_nc()
    shared = _prep_shared(inputs)
    in_maps = _build_maps(inputs, shared)
    from concourse.bass_utils import run_bass_kernel_spmd
    res = run_bass_kernel_spmd(nc, in_maps, core_ids=list(range(NCORES)))
    outs = []
    for s in range(NCORES):
        o = np.asarray(res.results[s]["out_o"])
        outs.append(np.ascontiguousarray(o.transpose(1, 0, 2)))
    return np.concatenate(outs, 0).astype(np.float32)
